# revision 1
# baseline (speedup 1.0000x reference)
"""AttentionHead kernel for 8 TRN2 NeuronCores.

Problem: q,k,v [4, 2048, 1024] f32; Wq/Wk/Wv [1024, 128]; out = softmax(
(qWq)(kWk)^T / sqrt(128)) @ (vWv)  -> [4, 2048, 128].

Sharding: core c = 2b+j owns batch b and query rows [1024j, 1024j+1024).
K/V projections are computed over the local half of the sequence and
exchanged between the two cores of a batch with pairwise AllGathers
(~0.5MB each), so every element of q/k/v is read from HBM exactly once.
The K gather is issued right after the k projection and overlaps the v
and q projection work; the V gather overlaps the q path.

On-chip layout: the PE contracts along partitions, so q/k/v tiles are
transposed on the tensor engine (fp32r is_transpose matmuls) into [h, s]
form.  Projections keep everything transposed (QT/KT/VT = [d, s]) with
the weight chunk as the stationary operand and N=512 moving (fp32r at
full rate).  VT is re-transposed to value [sk, d] tiles *before* the V
gather so both AllGather outputs are consumed with plain copies.
scoresT = KT-tiles.T @ QT -> [sk, sq]; softmax runs unnormalized (exp on
ACT with the 1/sqrt(d) scale folded in, normalization deferred);
contextT accumulates value-tiles.T @ expT -> [d, sq].  Rowsums
accumulate in PSUM via ones-vector matmuls (lhsT = ones[128,1])
interleaved with the attention matmuls; the final division by the
rowsum happens during the PSUM eviction of the re-transposed context
tiles.
"""

import os
from contextlib import ExitStack

# The kernel needs jax's axon TRN2 backend; a pinned cpu-only platform list
# (used by some harnesses for the jax reference) would hide the devices.
if os.environ.get("JAX_PLATFORMS") not in (None, "", "axon"):
    del os.environ["JAX_PLATFORMS"]

import numpy as np

import concourse.bass as bass
import concourse.tile as tile
import concourse.mybir as mybir
from concourse import bacc
from concourse.bass_utils import run_bass_kernel_spmd
from concourse.masks import make_identity

B, S, H, D = 4, 2048, 1024, 128
N_CORES = 8
SQ = 1024  # query rows per core
SKL = 1024  # local kv rows per core
SK = 2048  # kv rows per batch after allgather
HC = H // 128  # 8 chunks of the hidden dim
F32 = mybir.dt.float32
F32R = mybir.dt.float32r
BF16 = mybir.dt.bfloat16
SCALE = 1.0 / float(np.sqrt(np.float32(D)))
REPLICA_GROUPS = [[2 * b, 2 * b + 1] for b in range(B)]

_NC_CACHE = {}


def _f(ap):
    """View a float32r AP as plain fp32."""
    return ap.bitcast(F32)


def build():
    nc = bacc.Bacc(None, target_bir_lowering=False)
    q_d = nc.declare_dram_parameter("q", [SQ, H], BF16, isOutput=False)
    k_d = nc.declare_dram_parameter("k", [SKL, H], BF16, isOutput=False)
    v_d = nc.declare_dram_parameter("v", [SKL, H], BF16, isOutput=False)
    w_d = {
        "q": nc.declare_dram_parameter("wq", [H, D], BF16, isOutput=False),
        "k": nc.declare_dram_parameter("wk", [H, D], BF16, isOutput=False),
        "v": nc.declare_dram_parameter("wv", [H, D], BF16, isOutput=False),
    }
    b_d = {
        "q": nc.declare_dram_parameter("bq", [D], F32, isOutput=False),
        "k": nc.declare_dram_parameter("bk", [D], F32, isOutput=False),
        "v": nc.declare_dram_parameter("bv", [D], F32, isOutput=False),
    }
    sel_d = {
        0: nc.declare_dram_parameter("sel0", [1], F32, isOutput=False),
        1: nc.declare_dram_parameter("sel1", [1], F32, isOutput=False),
    }
    out_d = nc.declare_dram_parameter("out", [SQ, D], F32, isOutput=True)

    with tile.TileContext(nc) as tc, ExitStack() as top:
        const = top.enter_context(tc.tile_pool(name="const", bufs=1))
        # fp32 identity for the fp32 tail transposes, fp32r one for the
        # input-tile transposes (an fp32-written operand feeding an fp32r
        # matmul is rejected by the BIR verifier).
        identity = const.tile([128, 128], F32)
        make_identity(nc, identity)
        identity_r = const.tile([128, 128], F32R)
        nc.vector.tensor_copy(out=identity_r[:], in_=identity[:])
        identity_b = const.tile([128, 128], BF16)
        nc.vector.tensor_copy(out=identity_b[:], in_=identity[:])
        ones_f = const.tile([128, 1], F32)
        nc.vector.memset(ones_f[:], 1.0)
        ones_r = const.tile([128, 1], F32R)
        nc.vector.tensor_copy(out=ones_r[:], in_=ones_f[:])
        sel_sb = {}
        for r_i in (0, 1):
            sel_sb[r_i] = const.tile([128, 1], F32, name=f"sel{r_i}_sb")
            nc.gpsimd.dma_start(
                out=sel_sb[r_i][:], in_=sel_d[r_i][:].to_broadcast([128, 1])
            )

        wb_sb = {}
        b_sb = {}
        for n in ("k", "q", "v"):
            wb_sb[n] = const.tile([128, HC, D], BF16, name=f"w{n}b_sb")
            nc.sync.dma_start(
                out=wb_sb[n][:],
                in_=w_d[n][:].rearrange("(c p) d -> p c d", p=128),
            )
            b_sb[n] = const.tile([128, 1], F32, name=f"b{n}_sb")
            nc.sync.dma_start(out=b_sb[n][:], in_=b_d[n][:].unsqueeze(1))

        proj = top.enter_context(tc.tile_pool(name="proj", bufs=1))
        qt_sb = proj.tile([128, SQ], F32R)  # QT [d, sq]
        kth = [proj.tile([128, SKL], BF16, name=f"kth{r}") for r in (0, 1)]
        valh = [proj.tile([128, SKL], BF16, name=f"valh{r}") for r in (0, 1)]
        kt_rem = proj.tile([128, SKL], F32R)  # partner's KT half
        val_rem = proj.tile([128, SKL], F32R)  # partner's value half
        blend_tmp = proj.tile([128, SKL], F32R)
        ktl_sb = proj.tile([128, SKL], F32R)  # local KT half [d, skl]
        vtl_sb = proj.tile([128, SKL], F32R)  # local VT half [d, skl]
        vl_sb = proj.tile([128, SKL], F32R)  # local value rows [skl, d]

        dram = top.enter_context(tc.tile_pool(name="dram", bufs=1, space="DRAM"))
        cc_in_k = dram.tile([128, SKL], BF16)
        cc_out_k = dram.tile([256, SKL], BF16)
        cc_in_v = dram.tile([128, SKL], BF16)
        cc_out_v = dram.tile([256, SKL], BF16)
        rs_dram = dram.tile([SQ], F32)

        with ExitStack() as ph1:
            xin = ph1.enter_context(tc.tile_pool(name="xin", bufs=12))
            xt_ps = ph1.enter_context(tc.tile_pool(name="xt_ps", bufs=3, space="PSUM"))
            xt_sb = ph1.enter_context(tc.tile_pool(name="xt_sb", bufs=4))
            pj_ps = ph1.enter_context(tc.tile_pool(name="pj_ps", bufs=2, space="PSUM"))
            stage = ph1.enter_context(tc.tile_pool(name="stage", bufs=1))

            ev_flip = [0]

            def evict(out_ap, in_ap):
                # 2:1 DVE:ACT split of PSUM->SBUF copies (ACT also owns exp).
                if ev_flip[0] % 3 < 2:
                    nc.vector.tensor_copy(out=out_ap, in_=in_ap)
                else:
                    nc.scalar.activation(
                        out_ap, in_ap, mybir.ActivationFunctionType.Copy
                    )
                ev_flip[0] += 1

            def project(x_d, name, dest_ap):
                """dest_ap [128, 1024] <- f32r((x_d @ W + b)^T)."""
                for half in range(2):
                    rbs = []
                    for i in range(4):
                        rb = xin.tile([128, H], BF16, tag="xin")
                        r0 = (half * 4 + i) * 128
                        nc.sync.dma_start(out=rb[:], in_=x_d[r0 : r0 + 128, :])
                        rbs.append(rb)
                    pj = pj_ps.tile([128, 512], F32, tag="pj")
                    for c in range(HC):
                        tp = xt_ps.tile([128, 512], BF16, tag="tp")
                        for i in range(4):
                            nc.tensor.transpose(
                                tp[:, i * 128 : (i + 1) * 128],
                                rbs[i][:, c * 128 : (c + 1) * 128],
                                identity_b[:],
                            )
                        xts = xt_sb.tile([128, 512], BF16, tag="xts")
                        evict(xts[:], tp[:])
                        nc.tensor.matmul(
                            pj[:],
                            wb_sb[name][:, c, :],
                            xts[:],
                            start=(c == 0),
                            stop=(c == HC - 1),
                        )
                    nc.vector.tensor_scalar(
                        out=dest_ap[:, half * 512 : (half + 1) * 512],
                        in0=pj[:],
                        scalar1=b_sb[name][:],
                        scalar2=None,
                        op0=mybir.AluOpType.add,
                    )

            # ---- PE warm-up: busy the PE during the first DMAs so the HAM
            # clock-gate reaches 8/8 before the real transposes arrive ----
            warm_ps = xt_ps.tile([128, 512], F32R, tag="tp")
            for i in range(16):
                nc.tensor.transpose(
                    warm_ps[:, (i % 4) * 128 : (i % 4 + 1) * 128],
                    identity_r[:],
                    identity_r[:],
                )
            warm_sink = stage.tile([128, 128], F32R)
            nc.vector.tensor_copy(out=warm_sink[:], in_=warm_ps[:, 0:128])

            # ---- K path, then its allgather (overlaps v/q paths) ----
            project(k_d, "k", ktl_sb[:])
            ktl_b = stage.tile([128, SKL], BF16)
            nc.vector.tensor_copy(out=ktl_b[:], in_=_f(ktl_sb[:]))
            nc.gpsimd.dma_start(out=cc_in_k[:], in_=ktl_b[:])
            nc.gpsimd.collective_compute(
                "AllGather",
                mybir.AluOpType.bypass,
                ins=[cc_in_k[:].opt()],
                outs=[cc_out_k[:].opt()],
                replica_groups=REPLICA_GROUPS,
            )

            # ---- Q path (emitted before v so QT is ready early) ----
            project(q_d, "q", qt_sb[:])

            # ---- V path: project, re-transpose to value layout, allgather ----
            project(v_d, "v", vtl_sb[:])
            for g in range(2):
                tp = xt_ps.tile([128, 512], F32R, tag="tp")
                for i in range(4):
                    c = g * 4 + i
                    nc.tensor.transpose(
                        tp[:, i * 128 : (i + 1) * 128],
                        vtl_sb[:, c * 128 : (c + 1) * 128],
                        identity_r[:],
                    )
                evict(vl_sb[:, g * 512 : (g + 1) * 512], tp[:])
            vl_b = stage.tile([128, SKL], BF16)
            nc.vector.tensor_copy(out=vl_b[:], in_=_f(vl_sb[:]))
            nc.gpsimd.dma_start(out=cc_in_v[:], in_=vl_b[:])
            nc.gpsimd.collective_compute(
                "AllGather",
                mybir.AluOpType.bypass,
                ins=[cc_in_v[:].opt()],
                outs=[cc_out_v[:].opt()],
                replica_groups=REPLICA_GROUPS,
            )

            # ---- consume gathers: load halves, blend the partner half ----
            # remote = h0*sel0 + h1*sel1 with one-hot sel (exact x1/x0 mults);
            # sel differs per core via in_maps, the program stays SPMD.
            for r_i in range(2):
                nc.gpsimd.dma_start(
                    out=kth[r_i][:], in_=cc_out_k[128 * r_i : 128 * (r_i + 1), :]
                )
            nc.vector.tensor_scalar(
                out=blend_tmp[:], in0=kth[0][:], scalar1=sel_sb[0][:],
                scalar2=None, op0=mybir.AluOpType.mult,
            )
            nc.vector.scalar_tensor_tensor(
                out=kt_rem[:], in0=kth[1][:], scalar=sel_sb[1][:],
                in1=blend_tmp[:], op0=mybir.AluOpType.mult,
                op1=mybir.AluOpType.add,
            )
            for r_i in range(2):
                nc.gpsimd.dma_start(
                    out=valh[r_i][:], in_=cc_out_v[128 * r_i : 128 * (r_i + 1), :]
                )
            nc.vector.tensor_scalar(
                out=blend_tmp[:], in0=valh[0][:], scalar1=sel_sb[0][:],
                scalar2=None, op0=mybir.AluOpType.mult,
            )
            nc.vector.scalar_tensor_tensor(
                out=val_rem[:], in0=valh[1][:], scalar=sel_sb[1][:],
                in1=blend_tmp[:], op0=mybir.AluOpType.mult,
                op1=mybir.AluOpType.add,
            )

        # ---- attention phase ----
        with ExitStack() as ph2:
            sc_ps = ph2.enter_context(tc.tile_pool(name="sc_ps", bufs=2, space="PSUM"))
            ctx_ps = ph2.enter_context(tc.tile_pool(name="ctx_ps", bufs=1, space="PSUM"))
            rs_ps = ph2.enter_context(tc.tile_pool(name="rs_ps", bufs=1, space="PSUM"))
            att = ph2.enter_context(tc.tile_pool(name="att", bufs=16))

            fin = ph2.enter_context(tc.tile_pool(name="fin", bufs=1))
            rs_row = fin.tile([1, SQ], F32)
            rs8 = fin.tile([SQ // 128, 128], F32)
            recip = fin.tile([128, SQ // 128], F32)
            ctxt_sb = fin.tile([128, SQ], F32)
            out_sb = fin.tile([128, SQ // 128, D], F32)

            ctx = ctx_ps.tile([128, SQ], F32)  # contextT [d, sq] accumulator
            rs = rs_ps.tile([1, SQ], F32)  # rowsumT [1, sq] accumulator
            nck = SK // 128  # 16 sk chunks
            # chunk sources: first 8 local (no collective dependency), then
            # 8 from the blended partner half.  sk order is irrelevant: rowsum
            # and context are unordered sums over sk.
            k_src = [(ktl_sb, c) for c in range(8)] + [(kt_rem, c) for c in range(8)]
            v_src = [(vl_sb, c) for c in range(8)] + [(val_rem, c) for c in range(8)]
            exs = []
            for c in range(nck):
                kt_t, kc = k_src[c]
                sc = sc_ps.tile([128, SQ], F32, tag="sc")
                for hseg in range(2):
                    nc.tensor.matmul(
                        sc[:, hseg * 512 : (hseg + 1) * 512],
                        kt_t[:, kc * 128 : (kc + 1) * 128],
                        qt_sb[:, hseg * 512 : (hseg + 1) * 512],
                        start=True,
                        stop=True,
                    )
                ex = att.tile([128, SQ], F32R, tag="ex", name=f"ex{c}")
                nc.scalar.activation(
                    ex[:], sc[:], mybir.ActivationFunctionType.Exp, scale=SCALE
                )
                exs.append(ex)

                def ctx_mm(cc, hsegs=(0, 1)):
                    v_t, vc = v_src[cc]
                    for hseg in hsegs:
                        nc.tensor.matmul(
                            ctx[:, hseg * 512 : (hseg + 1) * 512],
                            v_t[:, vc * 128 : (vc + 1) * 128],
                            exs[cc][:, hseg * 512 : (hseg + 1) * 512],
                            start=(cc == 0),
                            stop=(cc == nck - 1),
                        )

                # interleave local ctx chunks into the remote-scores stream
                if c >= nck // 2:
                    ctx_mm(c - nck // 2)

            # rowsum matmuls run after the scores+exp loop over the retained
            # exp tiles: they never make the in-order PE stream wait on ACT
            for c in range(nck):
                for hseg in range(2):
                    nc.tensor.matmul(
                        rs[0:1, hseg * 512 : (hseg + 1) * 512],
                        ones_r[:],
                        exs[c][:, hseg * 512 : (hseg + 1) * 512],
                        start=(c == 0),
                        stop=(c == nck - 1),
                    )

            # rowsum fixup runs on DVE/DMA while the remote ctx matmuls are
            # still accumulating (the PE-side transpose stays in the tail)
            nc.vector.tensor_copy(out=rs_row[:], in_=rs[:])
            nc.gpsimd.dma_start(out=rs_dram[:].unsqueeze(0), in_=rs_row[:])
            nc.gpsimd.dma_start(
                out=rs8[:], in_=rs_dram[:].rearrange("(j p) -> j p", p=128)
            )

            # remote ctx: all of sq-half 0 first so its PSUM half finishes
            # (and can drain through the tail) while half 1 still accumulates
            for hseg in (0, 1):
                for c in range(nck // 2, nck):
                    ctx_mm(c, hsegs=(hseg,))

            # ---- tail: reciprocal, re-transpose context, normalize, out ----
            rs8_ps = sc_ps.tile([128, 512], F32, tag="sc")
            nc.tensor.transpose(
                rs8_ps[:, 0 : SQ // 128], rs8[:], identity[0 : SQ // 128, 0 : SQ // 128]
            )
            nc.vector.reciprocal(out=recip[:], in_=rs8_ps[:, 0 : SQ // 128])

            out_view = out_d[:].rearrange("(j p) d -> p j d", p=128)
            for g in range(SQ // 512):
                # per sq-half: evict, transpose, normalize, DMA — half 0's
                # chain overlaps half 1's ctx matmuls
                nc.vector.tensor_copy(
                    out=ctxt_sb[:, g * 512 : (g + 1) * 512],
                    in_=ctx[:, g * 512 : (g + 1) * 512],
                )
                tp = sc_ps.tile([128, 512], F32, tag="sc")
                for i in range(4):
                    j = g * 4 + i
                    nc.tensor.transpose(
                        tp[:, i * 128 : (i + 1) * 128],
                        ctxt_sb[:, j * 128 : (j + 1) * 128],
                        identity[:],
                    )
                for i in range(4):
                    j = g * 4 + i
                    nc.vector.tensor_scalar(
                        out=out_sb[:, j, :],
                        in0=tp[:, i * 128 : (i + 1) * 128],
                        scalar1=recip[:, j : j + 1],
                        scalar2=None,
                        op0=mybir.AluOpType.mult,
                    )
                nc.sync.dma_start(
                    out=out_view[:, g * 4 : (g + 1) * 4, :],
                    in_=out_sb[:, g * 4 : (g + 1) * 4, :],
                )

    nc.compile()
    return nc


def kernel(q, k, v, Wq, bq, Wk, bk, Wv, bv):
    import ml_dtypes

    bf16 = ml_dtypes.bfloat16
    q = np.ascontiguousarray(np.asarray(q, dtype=np.float32).astype(bf16))
    k = np.ascontiguousarray(np.asarray(k, dtype=np.float32).astype(bf16))
    v = np.ascontiguousarray(np.asarray(v, dtype=np.float32).astype(bf16))
    Wq = np.ascontiguousarray(np.asarray(Wq, dtype=np.float32).astype(bf16))
    Wk = np.ascontiguousarray(np.asarray(Wk, dtype=np.float32).astype(bf16))
    Wv = np.ascontiguousarray(np.asarray(Wv, dtype=np.float32).astype(bf16))
    bq = np.ascontiguousarray(np.asarray(bq, dtype=np.float32))
    bk = np.ascontiguousarray(np.asarray(bk, dtype=np.float32))
    bv = np.ascontiguousarray(np.asarray(bv, dtype=np.float32))

    if "nc" not in _NC_CACHE:
        _NC_CACHE["nc"] = build()
    nc = _NC_CACHE["nc"]

    half = S // 2  # 1024
    in_maps = []
    for c in range(N_CORES):
        b, j = c // 2, c % 2
        sl = slice(j * half, (j + 1) * half)
        in_maps.append(
            {
                "q": np.ascontiguousarray(q[b, sl]),
                "k": np.ascontiguousarray(k[b, sl]),
                "v": np.ascontiguousarray(v[b, sl]),
                "wq": Wq,
                "wk": Wk,
                "wv": Wv,
                "bq": bq,
                "bk": bk,
                "bv": bv,
                "sel0": np.array([1.0 if j == 1 else 0.0], np.float32),
                "sel1": np.array([1.0 if j == 0 else 0.0], np.float32),
            }
        )
    res = run_bass_kernel_spmd(nc, in_maps, list(range(N_CORES)))
    out = np.empty((B, S, D), dtype=np.float32)
    for c in range(N_CORES):
        b, j = c // 2, c % 2
        out[b, j * half : (j + 1) * half] = res.results[c]["out"]
    return out



# revision 3
# speedup vs baseline: 2.2207x; 2.2207x over previous
"""AttentionHead kernel for 8 TRN2 NeuronCores — v2 (no collectives).

Problem: q,k,v [4, 2048, 1024] f32; Wq/Wk/Wv [1024, 128]; out = softmax(
(qWq)(kWk)^T / sqrt(128)) @ (vWv)  -> [4, 2048, 128].

Sharding: core c = 2b+j owns batch b and query rows [1024j, 1024j+1024).
Instead of exchanging projected K/V halves between the two cores of a
batch (two 0.5MB AllGathers at ~28us each on the collective path), every
core projects the FULL K/V of its batch locally: the collective's fixed
cost dwarfs the extra 4MB of reads + ~7us of cheap bf16 projection
matmuls.

All activations are pre-transposed on the HOST (numpy) to [h, s] layout,
so no PE transposes / PSUM round-trips / DVE evictions are needed for
layout: the hidden dim is already on partitions for every projection
matmul.

On-chip dataflow (bf16 into the PE everywhere, fp32 PSUM accumulation):
  KT [d, sk]  = sum_c Wk-chunk.T @ ktT-chunk     (W stationary, kt moving)
  QT [d, sq]  likewise
  val[sk, d]  = sum_c vtT-chunk.T @ Wv-chunk     (vt stationary, W moving)
  scT[sk_c, sq] = KT-slice.T @ QT                (16 sk chunks)
  ex = exp(scale * scT)                          (ACT, bf16 out, kept in SBUF)
  ctx[sq_s, d] += ex-slice.T @ val[c]            (8 sq segs x 16 chunks)
  rs [sq_s, 1] += ex-slice.T @ ones              (rowsums, free-size-1 matmuls)
  out[sq_s, d] = ctx * (1/rs) + bv               (DVE/gpsimd eviction)

Q/K biases fold into the projection evictions (per-partition scalars on
d); the V bias commutes through the attention average and is added at
the output eviction (softmax rows sum to 1).  Junk "warm" matmuls plug
DMA-wait gaps in the PE stream so the p-state ramp never resets.
"""

import os
from contextlib import ExitStack

# The kernel needs jax's axon TRN2 backend; a pinned cpu-only platform list
# (used by some harnesses for the jax reference) would hide the devices.
if os.environ.get("JAX_PLATFORMS") not in (None, "", "axon"):
    del os.environ["JAX_PLATFORMS"]

import numpy as np

import concourse.bass as bass
import concourse.tile as tile
import concourse.mybir as mybir
from concourse import bacc
from concourse.bass_utils import run_bass_kernel_spmd

B, S, H, D = 4, 2048, 1024, 128
N_CORES = 8
SQ = 1024  # query rows per core
SK = 2048  # kv rows per batch (projected fully on both cores of the pair)
HC = H // 128  # 8 chunks of the hidden dim
NCK = SK // 128  # 16 sk chunks
NSEG = SQ // 128  # 8 sq segments
F32 = mybir.dt.float32
BF16 = mybir.dt.bfloat16
SCALE = 1.0 / float(np.sqrt(np.float32(D)))

_NC_CACHE = {}


def build():
    nc = bacc.Bacc(None, target_bir_lowering=False)
    qt_d = nc.declare_dram_parameter("qt", [H, SQ], BF16, isOutput=False)
    kt_d = nc.declare_dram_parameter("kt", [H, SK], BF16, isOutput=False)
    vt_d = nc.declare_dram_parameter("vt", [H, SK], BF16, isOutput=False)
    # packed weights: w[p, ((i, c, d))] = W_i[c*128 + p, d] for i in (q, k, v)
    w_d = nc.declare_dram_parameter("w", [128, 3 * HC * 128], BF16, isOutput=False)
    b_d = nc.declare_dram_parameter("b", [128, 3], F32, isOutput=False)
    bvr_d = nc.declare_dram_parameter("bvr", [128, 128], F32, isOutput=False)
    out_d = nc.declare_dram_parameter("out", [SQ, D], F32, isOutput=True)

    with tile.TileContext(nc) as tc, ExitStack() as top:
        const = top.enter_context(tc.tile_pool(name="const", bufs=1))
        w_sb = const.tile([128, 3 * HC * 128], BF16)
        b_sb = const.tile([128, 3], F32)
        bvr_sb = const.tile([128, 128], F32)
        ones_b = const.tile([128, 1], BF16)
        warm_w = const.tile([128, 128], BF16)
        warm_a = const.tile([128, 384], BF16)
        nc.vector.memset(ones_b[:], 1.0)
        nc.vector.memset(warm_w[:], 0.0)
        nc.vector.memset(warm_a[:], 0.0)

        def wsl(i, c):  # stationary weight slice [h-chunk, d]
            return w_sb[:, (i * HC + c) * 128 : (i * HC + c + 1) * 128]

        xin = top.enter_context(tc.tile_pool(name="xin", bufs=1))
        qt_sb = xin.tile([128, HC, SQ], BF16)
        kt_sb = xin.tile([128, HC, SK], BF16)
        vt_sb = xin.tile([128, HC, SK], BF16)

        res = top.enter_context(tc.tile_pool(name="res", bufs=1))
        QT = res.tile([128, SQ], BF16)  # [d, sq]
        KT = res.tile([128, SK], BF16)  # [d, sk]
        val = res.tile([128, NCK, 128], BF16)  # [sk-in-chunk, c, d]
        ex = res.tile([128, NCK, SQ], BF16)  # [sk-in-chunk, c, sq]
        recip_sb = res.tile([128, NSEG], F32)
        out_sb = res.tile([128, NSEG, 128], F32)

        # ---- input DMAs (SP/hwdge), ordered for pipeline liveness:
        # everything feeding the ACT-paced scores/exp stream (wk, kt, wq, qt)
        # goes first; vt only feeds PE-local context work and loads last.
        def load_w(i):
            sl = slice(i * HC * 128, (i + 1) * HC * 128)
            nc.sync.dma_start(out=w_sb[:, sl], in_=w_d[:, sl])

        def load_slab(dst, src, s0, s1):
            nc.sync.dma_start(
                out=dst[:, :, s0:s1],
                in_=src[:, s0:s1].rearrange("(c p) s -> p c s", p=128),
            )

        nc.sync.dma_start(out=w_sb[:, 0 : 2 * HC * 128], in_=w_d[:, 0 : 2 * HC * 128])
        nc.sync.dma_start(out=b_sb[:], in_=b_d[:])
        for i in range(4):
            load_slab(qt_sb, qt_d, i * 256, (i + 1) * 256)
        load_slab(kt_sb, kt_d, 0, 256)
        load_slab(kt_sb, kt_d, 256, 512)
        nc.sync.dma_start(
            out=w_sb[:, 2 * HC * 128 :], in_=w_d[:, 2 * HC * 128 :]
        )  # wv
        load_slab(kt_sb, kt_d, 512, 1024)
        load_slab(vt_sb, vt_d, 0, 512)
        load_slab(kt_sb, kt_d, 1024, 1536)
        load_slab(kt_sb, kt_d, 1536, 2048)
        load_slab(vt_sb, vt_d, 512, 1024)
        load_slab(vt_sb, vt_d, 1024, 1536)
        load_slab(vt_sb, vt_d, 1536, 2048)
        nc.sync.dma_start(out=bvr_sb[:], in_=bvr_d[:])

        with ExitStack() as ph:
            # PSUM budget (8 banks, bank-granular tiles):
            # pj 1 + sc 2x2 + ctx 2 + wr 1 (rs accumulator cols 0:8, junk
            # warm-up matmul region cols 128:384 — disjoint has_written
            # ranges in one bank).
            pj_ps = ph.enter_context(tc.tile_pool(name="pj_ps", bufs=1, space="PSUM"))
            sc_ps = ph.enter_context(tc.tile_pool(name="sc_ps", bufs=2, space="PSUM"))
            ctx_ps = ph.enter_context(tc.tile_pool(name="ctx_ps", bufs=1, space="PSUM"))
            wr_ps = ph.enter_context(tc.tile_pool(name="wr_ps", bufs=1, space="PSUM"))

            # two independent bank-tiles so tail evictions of the first
            # half overlap the PE finishing the second half
            ctxA = ctx_ps.tile([128, SQ // 2], F32)  # segs 0-3
            ctxB = ctx_ps.tile([128, SQ // 2], F32)  # segs 4-7
            wr = wr_ps.tile([128, 512], F32)  # rowsum accum cols 0:8

            def ctx_ap(s):
                t = ctxA if s < 4 else ctxB
                return t[:, (s % 4) * 128 : (s % 4 + 1) * 128]

            def warm(n):
                # junk matmuls: keep the PE busy through DMA waits so the
                # p-state ramp (3us to full clock) never restarts.
                for _ in range(n):
                    nc.tensor.matmul(
                        wr[:, 128:384], warm_w[:], warm_a[:, 0:256],
                        start=True, stop=True, skip_group_check=True,
                    )

            pgi = [0]

            def project_g(dst, wi, src, g, bias):
                # alternate between the pj bank and the spare region of the
                # wr bank so group g+1 accumulates while group g evicts (a
                # single bank would serialize every group behind its
                # eviction).  All projection start=True marks land before the
                # rowsum epoch opens (PE is in-order), so sharing wr is safe.
                i = pgi[0]
                pgi[0] += 1
                if i % 2 == 0:
                    pjt = pj_ps.tile([128, 256], F32, tag="pj", name=f"pj{i}")
                    pj = pjt[:]
                else:
                    pj = wr[:, 128:384]
                for c in range(HC):
                    nc.tensor.matmul(
                        pj,
                        wsl(wi, c),
                        src[:, c, g * 256 : (g + 1) * 256],
                        start=(c == 0),
                        stop=(c == HC - 1),
                        skip_group_check=True,
                    )
                eng = nc.vector
                eng.tensor_scalar(
                    out=dst[:, g * 256 : (g + 1) * 256], in0=pj, scalar1=bias,
                    scalar2=None, op0=mybir.AluOpType.add,
                )

            def vproj_pair(p, bank="pj"):  # sk chunks 2p, 2p+1
                # start=True zeroes the whole 2KB PSUM bank (zero region), so
                # only the FIRST matmul touching the bank starts; stop only on
                # the last.  Untouched-but-started bytes zero lazily on first
                # write (per-element has_written), so si=1 accumulates
                # correctly with start=False.  Pairs alternate between the pj
                # bank and the wr spare region so pair p+1 accumulates while
                # pair p evicts; all rowsum matmuls are emitted after the
                # last wr-bank start (PE is in-order), so sharing wr is safe.
                if bank == "pj":
                    vpt = pj_ps.tile([128, 256], F32, tag="pj", name=f"vp{p}")
                    vp = vpt[:]
                elif bank == "wr":
                    vp = wr[:, 128:384]
                else:  # recycle a scores bank (scores for this buf are done)
                    vpt = sc_ps.tile([128, SQ], F32, tag="sc", name=f"vp{p}")
                    vp = vpt[:, 0:256]
                for si in range(2):
                    seg = 2 * p + si
                    for c in range(HC):
                        nc.tensor.matmul(
                            vp[:, si * 128 : (si + 1) * 128],
                            vt_sb[:, c, seg * 128 : (seg + 1) * 128],
                            wsl(2, c),
                            start=(si == 0 and c == 0),
                            stop=(si == 1 and c == HC - 1),
                            skip_group_check=True,
                        )
                nc.vector.tensor_copy(out=val[:, 2 * p : 2 * p + 2, :], in_=vp)

            def scores(c):
                sc = sc_ps.tile([128, SQ], F32, tag="sc")
                for h in range(2):
                    nc.tensor.matmul(
                        sc[:, h * 512 : (h + 1) * 512],
                        KT[:, c * 128 : (c + 1) * 128],
                        QT[:, h * 512 : (h + 1) * 512],
                        start=True,
                        stop=True,
                    )
                nc.scalar.activation(
                    ex[:, c, :], sc[:], mybir.ActivationFunctionType.Exp, scale=SCALE
                )

            def ctx_chunk(c, first, last):
                # One start per 2KB PSUM bank per epoch (see vproj_pair): the
                # ctx tile spans 2 banks (segs 0-3 / 4-7).
                for s in range(NSEG):
                    nc.tensor.matmul(
                        ctx_ap(s),
                        ex[:, c, s * 128 : (s + 1) * 128],
                        val[:, c, :],
                        start=first and s % 4 == 0,
                        stop=last,
                        skip_group_check=True,
                    )

            def rs_chunk(c, first, last):
                # rowsums: free-size-1 matmuls, engine-time-free; deferred
                # until after the final wr-bank projection start.
                for s in range(NSEG):
                    nc.tensor.matmul(
                        wr[:, s : s + 1],
                        ex[:, c, s * 128 : (s + 1) * 128],
                        ones_b[:],
                        start=first and s == 0,
                        stop=last,
                        skip_group_check=True,
                    )

            # ---- PE stream (in emission order) ----
            # Interleave tuned to slab arrival: kproj groups right behind
            # their kt slabs, vproj pairs behind their vt slabs, ctx chunks
            # trailing the corresponding val evictions, scores ACT-paced.
            # wr's spare region hosts odd projection groups and vproj p1/p3/
            # p5; every wr start precedes the rowsum epoch (PE is in-order).
            warm(14)
            project_g(QT, 0, qt_sb, 0, b_sb[:, 0:1])
            warm(3)
            project_g(QT, 0, qt_sb, 1, b_sb[:, 0:1])
            warm(3)
            project_g(QT, 0, qt_sb, 2, b_sb[:, 0:1])
            warm(3)
            project_g(QT, 0, qt_sb, 3, b_sb[:, 0:1])
            warm(3)
            project_g(KT, 1, kt_sb, 0, b_sb[:, 1:2])
            warm(3)
            project_g(KT, 1, kt_sb, 1, b_sb[:, 1:2])
            for c in range(4):
                scores(c)
            project_g(KT, 1, kt_sb, 2, b_sb[:, 1:2])
            project_g(KT, 1, kt_sb, 3, b_sb[:, 1:2])
            scores(4)
            scores(5)
            vproj_pair(0, bank="pj")
            vproj_pair(1, bank="wr")
            ctx_chunk(0, first=True, last=False)
            scores(6)
            ctx_chunk(1, first=False, last=False)
            scores(7)
            project_g(KT, 1, kt_sb, 4, b_sb[:, 1:2])
            project_g(KT, 1, kt_sb, 5, b_sb[:, 1:2])
            ctx_chunk(2, first=False, last=False)
            scores(8)
            ctx_chunk(3, first=False, last=False)
            scores(9)
            project_g(KT, 1, kt_sb, 6, b_sb[:, 1:2])
            project_g(KT, 1, kt_sb, 7, b_sb[:, 1:2])
            scores(10)
            scores(11)
            vproj_pair(2, bank="pj")
            vproj_pair(3, bank="wr")
            scores(12)
            scores(13)
            scores(14)
            scores(15)
            vproj_pair(4, bank="pj")
            vproj_pair(5, bank="wr")  # last wr-bank start
            for c in range(4, 10):
                ctx_chunk(c, first=False, last=False)
            for c in range(14):
                rs_chunk(c, first=(c == 0), last=False)
            ctx_chunk(10, first=False, last=False)
            ctx_chunk(11, first=False, last=False)
            vproj_pair(6, bank="pj")
            vproj_pair(7, bank="sc")

            # ---- tail, seg-major: finish bank-A segs (chunks 12-15 + final
            # rowsums) first so their reciprocal/normalize/store chain runs
            # while the PE is still accumulating bank-B segs ----
            out_view = out_d[:].rearrange("(s p) d -> p s d", p=128)

            def finish_seg(s):
                for c in range(12, NCK):
                    nc.tensor.matmul(
                        ctx_ap(s),
                        ex[:, c, s * 128 : (s + 1) * 128],
                        val[:, c, :],
                        start=False,
                        stop=(c == NCK - 1),
                        skip_group_check=True,
                    )
                for c in range(14, NCK):
                    nc.tensor.matmul(
                        wr[:, s : s + 1],
                        ex[:, c, s * 128 : (s + 1) * 128],
                        ones_b[:],
                        start=False,
                        stop=(c == NCK - 1),
                        skip_group_check=True,
                    )

            def evict_out(s):
                eng = nc.vector
                eng.scalar_tensor_tensor(
                    out=out_sb[:, s, :],
                    in0=ctx_ap(s),
                    scalar=recip_sb[:, s : s + 1],
                    in1=bvr_sb[:],
                    op0=mybir.AluOpType.mult,
                    op1=mybir.AluOpType.add,
                )

            for s in range(4):
                finish_seg(s)
            nc.vector.reciprocal(out=recip_sb[:, 0:4], in_=wr[:, 0:4])
            for s in range(4):
                evict_out(s)
            nc.sync.dma_start(out=out_view[:, 0:4, :], in_=out_sb[:, 0:4, :])
            for s in range(4, 8):
                finish_seg(s)
            nc.vector.reciprocal(out=recip_sb[:, 4:8], in_=wr[:, 4:8])
            for s in range(4, 8):
                evict_out(s)
            nc.sync.dma_start(out=out_view[:, 4:8, :], in_=out_sb[:, 4:8, :])

    nc.compile()
    return nc


def _prep_inputs(q, k, v, Wq, bq, Wk, bk, Wv, bv):
    """Host-side packing: bf16 cast + [s,h]->[h,s] transposes + weight pack."""
    import ml_dtypes

    bf16 = ml_dtypes.bfloat16
    q = np.asarray(q, dtype=np.float32)
    k = np.asarray(k, dtype=np.float32)
    v = np.asarray(v, dtype=np.float32)
    w_pack = (
        np.stack(
            [np.asarray(Wq, np.float32), np.asarray(Wk, np.float32),
             np.asarray(Wv, np.float32)], 0
        )
        .reshape(3, HC, 128, D)
        .transpose(2, 0, 1, 3)
        .reshape(128, 3 * HC * D)
        .astype(bf16)
    )
    w_pack = np.ascontiguousarray(w_pack)
    b_pack = np.ascontiguousarray(
        np.stack(
            [np.asarray(bq, np.float32), np.asarray(bk, np.float32),
             np.asarray(bv, np.float32)], 1
        )
    )
    bv_rep = np.ascontiguousarray(
        np.broadcast_to(np.asarray(bv, np.float32), (128, D))
    )

    half = S // 2
    in_maps = []
    for c in range(N_CORES):
        b_i, j = c // 2, c % 2
        in_maps.append(
            {
                "qt": np.ascontiguousarray(
                    q[b_i, j * half : (j + 1) * half].T.astype(bf16)
                ),
                "kt": np.ascontiguousarray(k[b_i].T.astype(bf16)),
                "vt": np.ascontiguousarray(v[b_i].T.astype(bf16)),
                "w": w_pack,
                "b": b_pack,
                "bvr": bv_rep,
            }
        )
    return in_maps


def kernel(q, k, v, Wq, bq, Wk, bk, Wv, bv):
    if "nc" not in _NC_CACHE:
        _NC_CACHE["nc"] = build()
    nc = _NC_CACHE["nc"]

    in_maps = _prep_inputs(q, k, v, Wq, bq, Wk, bk, Wv, bv)
    res = run_bass_kernel_spmd(nc, in_maps, list(range(N_CORES)))
    half = S // 2
    out = np.empty((B, S, D), dtype=np.float32)
    for c in range(N_CORES):
        b_i, j = c // 2, c % 2
        out[b_i, j * half : (j + 1) * half] = res.results[c]["out"]
    return out


# revision 4
# speedup vs baseline: 2.2290x; 1.0037x over previous
"""AttentionHead kernel for 8 TRN2 NeuronCores — v2 (no collectives).

Problem: q,k,v [4, 2048, 1024] f32; Wq/Wk/Wv [1024, 128]; out = softmax(
(qWq)(kWk)^T / sqrt(128)) @ (vWv)  -> [4, 2048, 128].

Sharding: core c = 2b+j owns batch b and query rows [1024j, 1024j+1024).
Instead of exchanging projected K/V halves between the two cores of a
batch (two 0.5MB AllGathers at ~28us each on the collective path), every
core projects the FULL K/V of its batch locally: the collective's fixed
cost dwarfs the extra 4MB of reads + ~7us of cheap bf16 projection
matmuls.

All activations are pre-transposed on the HOST (numpy) to [h, s] layout,
so no PE transposes / PSUM round-trips / DVE evictions are needed for
layout: the hidden dim is already on partitions for every projection
matmul.

On-chip dataflow (bf16 into the PE everywhere, fp32 PSUM accumulation):
  KT [d, sk]  = sum_c Wk-chunk.T @ ktT-chunk     (W stationary, kt moving)
  QT [d, sq]  likewise
  val[sk, d]  = sum_c vtT-chunk.T @ Wv-chunk     (vt stationary, W moving)
  scT[sk_c, sq] = KT-slice.T @ QT                (16 sk chunks)
  ex = exp(scale * scT)                          (ACT, bf16 out, kept in SBUF)
  ctx[sq_s, d] += ex-slice.T @ val[c]            (8 sq segs x 16 chunks)
  rs [sq_s, 1] += ex-slice.T @ ones              (rowsums, free-size-1 matmuls)
  out[sq_s, d] = ctx * (1/rs) + bv               (DVE/gpsimd eviction)

Q/K biases fold into the projection evictions (per-partition scalars on
d); the V bias commutes through the attention average and is added at
the output eviction (softmax rows sum to 1).  Junk "warm" matmuls plug
DMA-wait gaps in the PE stream so the p-state ramp never resets.
"""

import os
from contextlib import ExitStack

# The kernel needs jax's axon TRN2 backend; a pinned cpu-only platform list
# (used by some harnesses for the jax reference) would hide the devices.
if os.environ.get("JAX_PLATFORMS") not in (None, "", "axon"):
    del os.environ["JAX_PLATFORMS"]

import numpy as np

import concourse.bass as bass
import concourse.tile as tile
import concourse.mybir as mybir
from concourse import bacc
from concourse.bass_utils import run_bass_kernel_spmd

B, S, H, D = 4, 2048, 1024, 128
N_CORES = 8
SQ = 1024  # query rows per core
SK = 2048  # kv rows per batch (projected fully on both cores of the pair)
HC = H // 128  # 8 chunks of the hidden dim
NCK = SK // 128  # 16 sk chunks
NSEG = SQ // 128  # 8 sq segments
F32 = mybir.dt.float32
BF16 = mybir.dt.bfloat16
SCALE = 1.0 / float(np.sqrt(np.float32(D)))

_NC_CACHE = {}


def build():
    nc = bacc.Bacc(None, target_bir_lowering=False)
    qt_d = nc.declare_dram_parameter("qt", [H, SQ], BF16, isOutput=False)
    kt_d = nc.declare_dram_parameter("kt", [H, SK], BF16, isOutput=False)
    vt_d = nc.declare_dram_parameter("vt", [H, SK], BF16, isOutput=False)
    # packed weights: w[p, ((i, c, d))] = W_i[c*128 + p, d] for i in (q, k, v)
    w_d = nc.declare_dram_parameter("w", [128, 3 * HC * 128], BF16, isOutput=False)
    b_d = nc.declare_dram_parameter("b", [128, 3], F32, isOutput=False)
    bvr_d = nc.declare_dram_parameter("bvr", [128, 4 * 128], F32, isOutput=False)
    out_d = nc.declare_dram_parameter("out", [SQ, D], F32, isOutput=True)

    with tile.TileContext(nc) as tc, ExitStack() as top:
        const = top.enter_context(tc.tile_pool(name="const", bufs=1))
        w_sb = const.tile([128, 3 * HC * 128], BF16)
        b_sb = const.tile([128, 3], F32)
        bvr_sb = const.tile([128, 4, 128], F32)
        ones_b = const.tile([128, 1], BF16)
        warm_w = const.tile([128, 128], BF16)
        warm_a = const.tile([128, 384], BF16)
        nc.vector.memset(ones_b[:], 1.0)
        nc.vector.memset(warm_w[:], 0.0)
        nc.vector.memset(warm_a[:], 0.0)

        def wsl(i, c):  # stationary weight slice [h-chunk, d]
            return w_sb[:, (i * HC + c) * 128 : (i * HC + c + 1) * 128]

        xin = top.enter_context(tc.tile_pool(name="xin", bufs=1))
        qt_sb = xin.tile([128, HC, SQ], BF16)
        kt_sb = xin.tile([128, HC, SK], BF16)
        vt_sb = xin.tile([128, HC, SK], BF16)

        res = top.enter_context(tc.tile_pool(name="res", bufs=1))
        QT = res.tile([128, SQ], BF16)  # [d, sq]
        KT = res.tile([128, SK], BF16)  # [d, sk]
        val = res.tile([128, NCK, 128], BF16)  # [sk-in-chunk, c, d]
        ex = res.tile([128, NCK, SQ], BF16)  # [sk-in-chunk, c, sq]
        recip_sb = res.tile([128, NSEG], F32)
        out_sb = res.tile([128, NSEG, 128], F32)

        # ---- input DMAs (SP/hwdge), ordered for pipeline liveness:
        # everything feeding the ACT-paced scores/exp stream (wk, kt, wq, qt)
        # goes first; vt only feeds PE-local context work and loads last.
        def load_w(i):
            sl = slice(i * HC * 128, (i + 1) * HC * 128)
            nc.sync.dma_start(out=w_sb[:, sl], in_=w_d[:, sl])

        def load_slab(dst, src, s0, s1):
            nc.sync.dma_start(
                out=dst[:, :, s0:s1],
                in_=src[:, s0:s1].rearrange("(c p) s -> p c s", p=128),
            )

        nc.sync.dma_start(out=w_sb[:, 0 : 2 * HC * 128], in_=w_d[:, 0 : 2 * HC * 128])
        nc.sync.dma_start(out=b_sb[:], in_=b_d[:])
        for i in range(4):
            load_slab(qt_sb, qt_d, i * 256, (i + 1) * 256)
        load_slab(kt_sb, kt_d, 0, 256)
        load_slab(kt_sb, kt_d, 256, 512)
        nc.sync.dma_start(
            out=w_sb[:, 2 * HC * 128 :], in_=w_d[:, 2 * HC * 128 :]
        )  # wv
        load_slab(kt_sb, kt_d, 512, 768)
        load_slab(kt_sb, kt_d, 768, 1024)
        load_slab(vt_sb, vt_d, 0, 512)
        load_slab(kt_sb, kt_d, 1024, 1280)
        load_slab(kt_sb, kt_d, 1280, 1536)
        load_slab(kt_sb, kt_d, 1536, 1792)
        load_slab(kt_sb, kt_d, 1792, 2048)
        load_slab(vt_sb, vt_d, 512, 1024)
        load_slab(vt_sb, vt_d, 1024, 1536)
        load_slab(vt_sb, vt_d, 1536, 2048)
        nc.sync.dma_start(
            out=bvr_sb[:], in_=bvr_d[:].rearrange("p (s d) -> p s d", d=128)
        )

        with ExitStack() as ph:
            # PSUM budget (8 banks, bank-granular tiles):
            # pj 1 + sc 2x2 + ctx 2 + wr 1 (rs accumulator cols 0:8, junk
            # warm-up matmul region cols 128:384 — disjoint has_written
            # ranges in one bank).
            pj_ps = ph.enter_context(tc.tile_pool(name="pj_ps", bufs=1, space="PSUM"))
            sc_ps = ph.enter_context(tc.tile_pool(name="sc_ps", bufs=2, space="PSUM"))
            ctx_ps = ph.enter_context(tc.tile_pool(name="ctx_ps", bufs=1, space="PSUM"))
            wr_ps = ph.enter_context(tc.tile_pool(name="wr_ps", bufs=1, space="PSUM"))

            # two independent bank-tiles so tail evictions of the first
            # half overlap the PE finishing the second half
            ctxA = ctx_ps.tile([128, SQ // 2], F32)  # segs 0-3
            ctxB = ctx_ps.tile([128, SQ // 2], F32)  # segs 4-7
            wr = wr_ps.tile([128, 512], F32)  # rowsum accum cols 0:8

            def ctx_ap(s):
                t = ctxA if s < 4 else ctxB
                return t[:, (s % 4) * 128 : (s % 4 + 1) * 128]

            def warm(n):
                # junk matmuls: keep the PE busy through DMA waits so the
                # p-state ramp (3us to full clock) never restarts.
                for _ in range(n):
                    nc.tensor.matmul(
                        wr[:, 128:384], warm_w[:], warm_a[:, 0:256],
                        start=True, stop=True, skip_group_check=True,
                    )

            pgi = [0]

            def project_g(dst, wi, src, g, bias):
                # alternate between the pj bank and the spare region of the
                # wr bank so group g+1 accumulates while group g evicts (a
                # single bank would serialize every group behind its
                # eviction).  All projection start=True marks land before the
                # rowsum epoch opens (PE is in-order), so sharing wr is safe.
                i = pgi[0]
                pgi[0] += 1
                if i % 2 == 0:
                    pjt = pj_ps.tile([128, 256], F32, tag="pj", name=f"pj{i}")
                    pj = pjt[:]
                else:
                    pj = wr[:, 128:384]
                for c in range(HC):
                    nc.tensor.matmul(
                        pj,
                        wsl(wi, c),
                        src[:, c, g * 256 : (g + 1) * 256],
                        start=(c == 0),
                        stop=(c == HC - 1),
                        skip_group_check=True,
                    )
                eng = nc.vector
                eng.tensor_scalar(
                    out=dst[:, g * 256 : (g + 1) * 256], in0=pj, scalar1=bias,
                    scalar2=None, op0=mybir.AluOpType.add,
                )

            def vproj_pair(p, bank="pj"):  # sk chunks 2p, 2p+1
                # start=True zeroes the whole 2KB PSUM bank (zero region), so
                # only the FIRST matmul touching the bank starts; stop only on
                # the last.  Untouched-but-started bytes zero lazily on first
                # write (per-element has_written), so si=1 accumulates
                # correctly with start=False.  Pairs alternate between the pj
                # bank and the wr spare region so pair p+1 accumulates while
                # pair p evicts; all rowsum matmuls are emitted after the
                # last wr-bank start (PE is in-order), so sharing wr is safe.
                if bank == "pj":
                    vpt = pj_ps.tile([128, 256], F32, tag="pj", name=f"vp{p}")
                    vp = vpt[:]
                elif bank == "wr":
                    vp = wr[:, 128:384]
                else:  # recycle a scores bank (scores for this buf are done)
                    vpt = sc_ps.tile([128, SQ], F32, tag="sc", name=f"vp{p}")
                    vp = vpt[:, 0:256]
                for si in range(2):
                    seg = 2 * p + si
                    for c in range(HC):
                        nc.tensor.matmul(
                            vp[:, si * 128 : (si + 1) * 128],
                            vt_sb[:, c, seg * 128 : (seg + 1) * 128],
                            wsl(2, c),
                            start=(si == 0 and c == 0),
                            stop=(si == 1 and c == HC - 1),
                            skip_group_check=True,
                        )
                nc.vector.tensor_copy(out=val[:, 2 * p : 2 * p + 2, :], in_=vp)

            def scores(c):
                sc = sc_ps.tile([128, SQ], F32, tag="sc")
                for h in range(2):
                    nc.tensor.matmul(
                        sc[:, h * 512 : (h + 1) * 512],
                        KT[:, c * 128 : (c + 1) * 128],
                        QT[:, h * 512 : (h + 1) * 512],
                        start=True,
                        stop=True,
                    )
                nc.scalar.activation(
                    ex[:, c, :], sc[:], mybir.ActivationFunctionType.Exp, scale=SCALE
                )

            def ctx_chunk(c, first, last):
                # One start per 2KB PSUM bank per epoch (see vproj_pair): the
                # ctx tile spans 2 banks (segs 0-3 / 4-7).
                for s in range(NSEG):
                    nc.tensor.matmul(
                        ctx_ap(s),
                        ex[:, c, s * 128 : (s + 1) * 128],
                        val[:, c, :],
                        start=first and s % 4 == 0,
                        stop=last,
                        skip_group_check=True,
                    )

            def rs_chunk(c, first, last):
                # rowsums: free-size-1 matmuls, engine-time-free; deferred
                # until after the final wr-bank projection start.
                for s in range(NSEG):
                    nc.tensor.matmul(
                        wr[:, s : s + 1],
                        ex[:, c, s * 128 : (s + 1) * 128],
                        ones_b[:],
                        start=first and s == 0,
                        stop=last,
                        skip_group_check=True,
                    )

            # ---- PE stream (in emission order) ----
            # Interleave tuned to slab arrival: kproj groups right behind
            # their kt slabs, vproj pairs behind their vt slabs, ctx chunks
            # trailing the corresponding val evictions, scores ACT-paced.
            # wr's spare region hosts odd projection groups and vproj p1/p3/
            # p5; every wr start precedes the rowsum epoch (PE is in-order).
            warm(14)
            project_g(QT, 0, qt_sb, 0, b_sb[:, 0:1])
            warm(3)
            project_g(QT, 0, qt_sb, 1, b_sb[:, 0:1])
            warm(3)
            project_g(QT, 0, qt_sb, 2, b_sb[:, 0:1])
            warm(3)
            project_g(QT, 0, qt_sb, 3, b_sb[:, 0:1])
            warm(3)
            project_g(KT, 1, kt_sb, 0, b_sb[:, 1:2])
            warm(3)
            project_g(KT, 1, kt_sb, 1, b_sb[:, 1:2])
            for c in range(4):
                scores(c)
            project_g(KT, 1, kt_sb, 2, b_sb[:, 1:2])
            project_g(KT, 1, kt_sb, 3, b_sb[:, 1:2])
            scores(4)
            scores(5)
            vproj_pair(0, bank="pj")
            vproj_pair(1, bank="wr")
            ctx_chunk(0, first=True, last=False)
            scores(6)
            ctx_chunk(1, first=False, last=False)
            scores(7)
            project_g(KT, 1, kt_sb, 4, b_sb[:, 1:2])
            project_g(KT, 1, kt_sb, 5, b_sb[:, 1:2])
            ctx_chunk(2, first=False, last=False)
            scores(8)
            ctx_chunk(3, first=False, last=False)
            scores(9)
            project_g(KT, 1, kt_sb, 6, b_sb[:, 1:2])
            project_g(KT, 1, kt_sb, 7, b_sb[:, 1:2])
            scores(10)
            scores(11)
            vproj_pair(2, bank="pj")
            vproj_pair(3, bank="wr")
            scores(12)
            scores(13)
            scores(14)
            scores(15)
            vproj_pair(4, bank="pj")
            vproj_pair(5, bank="wr")  # last wr-bank start
            for c in range(4, 10):
                ctx_chunk(c, first=False, last=False)
            for c in range(14):
                rs_chunk(c, first=(c == 0), last=False)
            ctx_chunk(10, first=False, last=False)
            ctx_chunk(11, first=False, last=False)
            vproj_pair(6, bank="pj")
            vproj_pair(7, bank="sc")

            # ---- tail, seg-major: finish bank-A segs (chunks 12-15 + final
            # rowsums) first so their reciprocal/normalize/store chain runs
            # while the PE is still accumulating bank-B segs ----
            out_view = out_d[:].rearrange("(s p) d -> p s d", p=128)

            def finish_seg(s):
                for c in range(12, NCK):
                    nc.tensor.matmul(
                        ctx_ap(s),
                        ex[:, c, s * 128 : (s + 1) * 128],
                        val[:, c, :],
                        start=False,
                        stop=(c == NCK - 1),
                        skip_group_check=True,
                    )
                for c in range(14, NCK):
                    nc.tensor.matmul(
                        wr[:, s : s + 1],
                        ex[:, c, s * 128 : (s + 1) * 128],
                        ones_b[:],
                        start=False,
                        stop=(c == NCK - 1),
                        skip_group_check=True,
                    )

            def evict_half(h):
                for s in range(4 * h, 4 * h + 4):
                    nc.vector.scalar_tensor_tensor(
                        out=out_sb[:, s, :],
                        in0=ctx_ap(s),
                        scalar=recip_sb[:, s : s + 1],
                        in1=bvr_sb[:, s % 4, :],
                        op0=mybir.AluOpType.mult,
                        op1=mybir.AluOpType.add,
                    )
                nc.sync.dma_start(
                    out=out_view[:, 4 * h : 4 * h + 4, :],
                    in_=out_sb[:, 4 * h : 4 * h + 4, :],
                )

            for s in range(4):
                finish_seg(s)
            nc.vector.reciprocal(out=recip_sb[:, 0:4], in_=wr[:, 0:4])
            evict_half(0)
            for s in range(4, 8):
                finish_seg(s)
            nc.vector.reciprocal(out=recip_sb[:, 4:8], in_=wr[:, 4:8])
            evict_half(1)

    nc.compile()
    return nc


def _prep_inputs(q, k, v, Wq, bq, Wk, bk, Wv, bv):
    """Host-side packing: bf16 cast + [s,h]->[h,s] transposes + weight pack."""
    import ml_dtypes

    bf16 = ml_dtypes.bfloat16
    q = np.asarray(q, dtype=np.float32)
    k = np.asarray(k, dtype=np.float32)
    v = np.asarray(v, dtype=np.float32)
    w_pack = (
        np.stack(
            [np.asarray(Wq, np.float32), np.asarray(Wk, np.float32),
             np.asarray(Wv, np.float32)], 0
        )
        .reshape(3, HC, 128, D)
        .transpose(2, 0, 1, 3)
        .reshape(128, 3 * HC * D)
        .astype(bf16)
    )
    w_pack = np.ascontiguousarray(w_pack)
    b_pack = np.ascontiguousarray(
        np.stack(
            [np.asarray(bq, np.float32), np.asarray(bk, np.float32),
             np.asarray(bv, np.float32)], 1
        )
    )
    bv_rep = np.ascontiguousarray(
        np.broadcast_to(np.asarray(bv, np.float32), (128, 4, D)).reshape(128, 4 * D)
    )

    half = S // 2
    in_maps = []
    for c in range(N_CORES):
        b_i, j = c // 2, c % 2
        in_maps.append(
            {
                "qt": np.ascontiguousarray(
                    q[b_i, j * half : (j + 1) * half].T.astype(bf16)
                ),
                "kt": np.ascontiguousarray(k[b_i].T.astype(bf16)),
                "vt": np.ascontiguousarray(v[b_i].T.astype(bf16)),
                "w": w_pack,
                "b": b_pack,
                "bvr": bv_rep,
            }
        )
    return in_maps


def kernel(q, k, v, Wq, bq, Wk, bk, Wv, bv):
    if "nc" not in _NC_CACHE:
        _NC_CACHE["nc"] = build()
    nc = _NC_CACHE["nc"]

    in_maps = _prep_inputs(q, k, v, Wq, bq, Wk, bk, Wv, bv)
    res = run_bass_kernel_spmd(nc, in_maps, list(range(N_CORES)))
    half = S // 2
    out = np.empty((B, S, D), dtype=np.float32)
    for c in range(N_CORES):
        b_i, j = c // 2, c % 2
        out[b_i, j * half : (j + 1) * half] = res.results[c]["out"]
    return out


# revision 5
# speedup vs baseline: 2.2507x; 1.0097x over previous
"""AttentionHead kernel for 8 TRN2 NeuronCores — v2 (no collectives).

Problem: q,k,v [4, 2048, 1024] f32; Wq/Wk/Wv [1024, 128]; out = softmax(
(qWq)(kWk)^T / sqrt(128)) @ (vWv)  -> [4, 2048, 128].

Sharding: core c = 2b+j owns batch b and query rows [1024j, 1024j+1024).
Instead of exchanging projected K/V halves between the two cores of a
batch (two 0.5MB AllGathers at ~28us each on the collective path), every
core projects the FULL K/V of its batch locally: the collective's fixed
cost dwarfs the extra 4MB of reads + ~7us of cheap bf16 projection
matmuls.

All activations are pre-transposed on the HOST (numpy) to [h, s] layout,
so no PE transposes / PSUM round-trips / DVE evictions are needed for
layout: the hidden dim is already on partitions for every projection
matmul.

On-chip dataflow (bf16 into the PE everywhere, fp32 PSUM accumulation):
  KT [d, sk]  = sum_c Wk-chunk.T @ ktT-chunk     (W stationary, kt moving)
  QT [d, sq]  likewise
  val[sk, d]  = sum_c vtT-chunk.T @ Wv-chunk     (vt stationary, W moving)
  scT[sk_c, sq] = KT-slice.T @ QT                (16 sk chunks)
  ex = exp(scale * scT)                          (ACT, bf16 out, kept in SBUF)
  ctx[sq_s, d] += ex-slice.T @ val[c]            (8 sq segs x 16 chunks)
  rs [sq_s, 1] += ex-slice.T @ ones              (rowsums, free-size-1 matmuls)
  out[sq_s, d] = ctx * (1/rs) + bv               (DVE/gpsimd eviction)

Q/K biases fold into the projection evictions (per-partition scalars on
d); the V bias commutes through the attention average and is added at
the output eviction (softmax rows sum to 1).  Junk "warm" matmuls plug
DMA-wait gaps in the PE stream so the p-state ramp never resets.
"""

import os
from contextlib import ExitStack

# The kernel needs jax's axon TRN2 backend; a pinned cpu-only platform list
# (used by some harnesses for the jax reference) would hide the devices.
if os.environ.get("JAX_PLATFORMS") not in (None, "", "axon"):
    del os.environ["JAX_PLATFORMS"]

import numpy as np

import concourse.bass as bass
import concourse.tile as tile
import concourse.mybir as mybir
from concourse import bacc
from concourse.bass_utils import run_bass_kernel_spmd

B, S, H, D = 4, 2048, 1024, 128
N_CORES = 8
SQ = 1024  # query rows per core
SK = 2048  # kv rows per batch (projected fully on both cores of the pair)
HC = H // 128  # 8 chunks of the hidden dim
NCK = SK // 128  # 16 sk chunks
NSEG = SQ // 128  # 8 sq segments
F32 = mybir.dt.float32
BF16 = mybir.dt.bfloat16
SCALE = 1.0 / float(np.sqrt(np.float32(D)))

_NC_CACHE = {}


def build():
    nc = bacc.Bacc(None, target_bir_lowering=False)
    qt_d = nc.declare_dram_parameter("qt", [H, SQ], BF16, isOutput=False)
    kt_d = nc.declare_dram_parameter("kt", [H, SK], BF16, isOutput=False)
    vt_d = nc.declare_dram_parameter("vt", [H, SK], BF16, isOutput=False)
    # packed weights: w[p, ((i, c, d))] = W_i[c*128 + p, d] for i in (q, k, v)
    w_d = nc.declare_dram_parameter("w", [128, 3 * HC * 128], BF16, isOutput=False)
    b_d = nc.declare_dram_parameter("b", [128, 3], F32, isOutput=False)
    bvr_d = nc.declare_dram_parameter("bvr", [128, 4 * 128], F32, isOutput=False)
    out_d = nc.declare_dram_parameter("out", [SQ, D], F32, isOutput=True)

    with tile.TileContext(nc) as tc, ExitStack() as top:
        const = top.enter_context(tc.tile_pool(name="const", bufs=1))
        w_sb = const.tile([128, 3 * HC * 128], BF16)
        b_sb = const.tile([128, 3], F32)
        bvr_sb = const.tile([128, 4, 128], F32)
        ones_b = const.tile([128, 1], BF16)
        warm_w = const.tile([128, 128], BF16)
        warm_a = const.tile([128, 384], BF16)
        nc.vector.memset(ones_b[:], 1.0)
        nc.vector.memset(warm_w[:], 0.0)
        nc.vector.memset(warm_a[:], 0.0)

        def wsl(i, c):  # stationary weight slice [h-chunk, d]
            return w_sb[:, (i * HC + c) * 128 : (i * HC + c + 1) * 128]

        xin = top.enter_context(tc.tile_pool(name="xin", bufs=1))
        qt_sb = xin.tile([128, HC, SQ], BF16)
        kt_sb = xin.tile([128, HC, SK], BF16)
        vt_sb = xin.tile([128, HC, SK], BF16)

        res = top.enter_context(tc.tile_pool(name="res", bufs=1))
        QT = res.tile([128, SQ], BF16)  # [d, sq]
        KT = res.tile([128, SK], BF16)  # [d, sk]
        val = res.tile([128, NCK, 128], BF16)  # [sk-in-chunk, c, d]
        ex = res.tile([128, NCK, SQ], BF16)  # [sk-in-chunk, c, sq]
        recip_sb = res.tile([128, NSEG], F32)
        out_sb = res.tile([128, NSEG, 128], F32)

        # ---- input DMAs (SP/hwdge), ordered for pipeline liveness:
        # everything feeding the ACT-paced scores/exp stream (wk, kt, wq, qt)
        # goes first; vt only feeds PE-local context work and loads last.
        def load_w(i):
            sl = slice(i * HC * 128, (i + 1) * HC * 128)
            nc.sync.dma_start(out=w_sb[:, sl], in_=w_d[:, sl])

        def load_slab(dst, src, s0, s1):
            nc.sync.dma_start(
                out=dst[:, :, s0:s1],
                in_=src[:, s0:s1].rearrange("(c p) s -> p c s", p=128),
            )

        nc.sync.dma_start(out=w_sb[:, 0 : 2 * HC * 128], in_=w_d[:, 0 : 2 * HC * 128])
        nc.sync.dma_start(out=b_sb[:], in_=b_d[:])
        for i in range(4):
            load_slab(qt_sb, qt_d, i * 256, (i + 1) * 256)
        load_slab(kt_sb, kt_d, 0, 256)
        load_slab(kt_sb, kt_d, 256, 512)
        nc.sync.dma_start(
            out=w_sb[:, 2 * HC * 128 :], in_=w_d[:, 2 * HC * 128 :]
        )  # wv
        load_slab(kt_sb, kt_d, 512, 768)
        load_slab(kt_sb, kt_d, 768, 1024)
        load_slab(vt_sb, vt_d, 0, 512)
        load_slab(kt_sb, kt_d, 1024, 1280)
        load_slab(kt_sb, kt_d, 1280, 1536)
        load_slab(kt_sb, kt_d, 1536, 1792)
        load_slab(kt_sb, kt_d, 1792, 2048)
        load_slab(vt_sb, vt_d, 512, 1024)
        load_slab(vt_sb, vt_d, 1024, 1536)
        load_slab(vt_sb, vt_d, 1536, 2048)
        nc.sync.dma_start(
            out=bvr_sb[:], in_=bvr_d[:].rearrange("p (s d) -> p s d", d=128)
        )

        with ExitStack() as ph:
            # PSUM budget (8 banks, bank-granular tiles):
            # pj 1 + sc 2x2 + ctx 2 + wr 1 (rs accumulator cols 0:8, junk
            # warm-up matmul region cols 128:384 — disjoint has_written
            # ranges in one bank).
            pj_ps = ph.enter_context(tc.tile_pool(name="pj_ps", bufs=1, space="PSUM"))
            sc_ps = ph.enter_context(tc.tile_pool(name="sc_ps", bufs=2, space="PSUM"))
            ctx_ps = ph.enter_context(tc.tile_pool(name="ctx_ps", bufs=1, space="PSUM"))
            wr_ps = ph.enter_context(tc.tile_pool(name="wr_ps", bufs=1, space="PSUM"))

            # two independent bank-tiles so tail evictions of the first
            # half overlap the PE finishing the second half
            ctxA = ctx_ps.tile([128, SQ // 2], F32)  # segs 0-3
            ctxB = ctx_ps.tile([128, SQ // 2], F32)  # segs 4-7
            wr = wr_ps.tile([128, 512], F32)  # rowsum accum cols 0:8

            def ctx_ap(s):
                t = ctxA if s < 4 else ctxB
                return t[:, (s % 4) * 128 : (s % 4 + 1) * 128]

            def warm(n):
                # junk matmuls: keep the PE busy through DMA waits so the
                # p-state ramp (3us to full clock) never restarts.
                for _ in range(n):
                    nc.tensor.matmul(
                        wr[:, 128:384], warm_w[:], warm_a[:, 0:256],
                        start=True, stop=True, skip_group_check=True,
                    )

            pgi = [0]

            def project_g(dst, wi, src, g, bias):
                # alternate between the pj bank and the spare region of the
                # wr bank so group g+1 accumulates while group g evicts (a
                # single bank would serialize every group behind its
                # eviction).  All projection start=True marks land before the
                # rowsum epoch opens (PE is in-order), so sharing wr is safe.
                i = pgi[0]
                pgi[0] += 1
                if i % 2 == 0:
                    pjt = pj_ps.tile([128, 256], F32, tag="pj", name=f"pj{i}")
                    pj = pjt[:]
                else:
                    pj = wr[:, 128:384]
                for c in range(HC):
                    nc.tensor.matmul(
                        pj,
                        wsl(wi, c),
                        src[:, c, g * 256 : (g + 1) * 256],
                        start=(c == 0),
                        stop=(c == HC - 1),
                        skip_group_check=True,
                    )
                eng = nc.vector
                eng.tensor_scalar(
                    out=dst[:, g * 256 : (g + 1) * 256], in0=pj, scalar1=bias,
                    scalar2=None, op0=mybir.AluOpType.add,
                )

            def vproj_pair(p, bank="pj"):  # sk chunks 2p, 2p+1
                # start=True zeroes the whole 2KB PSUM bank (zero region), so
                # only the FIRST matmul touching the bank starts; stop only on
                # the last.  Untouched-but-started bytes zero lazily on first
                # write (per-element has_written), so si=1 accumulates
                # correctly with start=False.  Pairs alternate between the pj
                # bank and the wr spare region so pair p+1 accumulates while
                # pair p evicts; all rowsum matmuls are emitted after the
                # last wr-bank start (PE is in-order), so sharing wr is safe.
                if bank == "pj":
                    vpt = pj_ps.tile([128, 256], F32, tag="pj", name=f"vp{p}")
                    vp = vpt[:]
                elif bank == "wr":
                    vp = wr[:, 128:384]
                else:  # recycle a scores bank (scores for this buf are done)
                    vpt = sc_ps.tile([128, SQ], F32, tag="sc", name=f"vp{p}")
                    vp = vpt[:, 0:256]
                for si in range(2):
                    seg = 2 * p + si
                    for c in range(HC):
                        nc.tensor.matmul(
                            vp[:, si * 128 : (si + 1) * 128],
                            vt_sb[:, c, seg * 128 : (seg + 1) * 128],
                            wsl(2, c),
                            start=(si == 0 and c == 0),
                            stop=(si == 1 and c == HC - 1),
                            skip_group_check=True,
                        )
                nc.vector.tensor_copy(out=val[:, 2 * p : 2 * p + 2, :], in_=vp)

            def scores(c):
                sc = sc_ps.tile([128, SQ], F32, tag="sc")
                for h in range(2):
                    nc.tensor.matmul(
                        sc[:, h * 512 : (h + 1) * 512],
                        KT[:, c * 128 : (c + 1) * 128],
                        QT[:, h * 512 : (h + 1) * 512],
                        start=True,
                        stop=True,
                    )
                nc.scalar.activation(
                    ex[:, c, :], sc[:], mybir.ActivationFunctionType.Exp, scale=SCALE
                )

            def ctx_chunk(c, first, last):
                # One start per 2KB PSUM bank per epoch (see vproj_pair): the
                # ctx tile spans 2 banks (segs 0-3 / 4-7).
                for s in range(NSEG):
                    nc.tensor.matmul(
                        ctx_ap(s),
                        ex[:, c, s * 128 : (s + 1) * 128],
                        val[:, c, :],
                        start=first and s % 4 == 0,
                        stop=last,
                        skip_group_check=True,
                    )

            def rs_chunk(c, first, last):
                # rowsums: free-size-1 matmuls, engine-time-free; deferred
                # until after the final wr-bank projection start.
                for s in range(NSEG):
                    nc.tensor.matmul(
                        wr[:, s : s + 1],
                        ex[:, c, s * 128 : (s + 1) * 128],
                        ones_b[:],
                        start=first and s == 0,
                        stop=last,
                        skip_group_check=True,
                    )

            # ---- PE stream (in emission order) ----
            # Interleave tuned to slab arrival: kproj groups right behind
            # their kt slabs, vproj pairs behind their vt slabs, ctx chunks
            # trailing the corresponding val evictions, scores ACT-paced.
            # wr's spare region hosts odd projection groups and vproj p1/p3/
            # p5; every wr start precedes the rowsum epoch (PE is in-order).
            warm(14)
            project_g(QT, 0, qt_sb, 0, b_sb[:, 0:1])
            warm(3)
            project_g(QT, 0, qt_sb, 1, b_sb[:, 0:1])
            warm(3)
            project_g(QT, 0, qt_sb, 2, b_sb[:, 0:1])
            warm(3)
            project_g(QT, 0, qt_sb, 3, b_sb[:, 0:1])
            warm(3)
            project_g(KT, 1, kt_sb, 0, b_sb[:, 1:2])
            warm(3)
            project_g(KT, 1, kt_sb, 1, b_sb[:, 1:2])
            for c in range(4):
                scores(c)
            project_g(KT, 1, kt_sb, 2, b_sb[:, 1:2])
            project_g(KT, 1, kt_sb, 3, b_sb[:, 1:2])
            scores(4)
            scores(5)
            vproj_pair(0, bank="pj")
            vproj_pair(1, bank="wr")
            ctx_chunk(0, first=True, last=False)
            scores(6)
            ctx_chunk(1, first=False, last=False)
            scores(7)
            project_g(KT, 1, kt_sb, 4, b_sb[:, 1:2])
            project_g(KT, 1, kt_sb, 5, b_sb[:, 1:2])
            ctx_chunk(2, first=False, last=False)
            scores(8)
            ctx_chunk(3, first=False, last=False)
            scores(9)
            project_g(KT, 1, kt_sb, 6, b_sb[:, 1:2])
            project_g(KT, 1, kt_sb, 7, b_sb[:, 1:2])
            scores(10)
            scores(11)
            vproj_pair(2, bank="pj")
            vproj_pair(3, bank="wr")
            scores(12)
            scores(13)
            scores(14)
            scores(15)
            vproj_pair(4, bank="pj")
            vproj_pair(5, bank="wr")  # last wr-bank start
            for c in range(4, 10):
                ctx_chunk(c, first=False, last=False)
            for c in range(14):
                rs_chunk(c, first=(c == 0), last=False)
            ctx_chunk(10, first=False, last=False)
            ctx_chunk(11, first=False, last=False)
            vproj_pair(6, bank="pj")
            vproj_pair(7, bank="sc")

            # ---- tail, seg-major: finish bank-A segs (chunks 12-15 + final
            # rowsums) first so their reciprocal/normalize/store chain runs
            # while the PE is still accumulating bank-B segs ----
            out_view = out_d[:].rearrange("(s p) d -> p s d", p=128)

            def finish_seg(s):
                for c in range(12, NCK):
                    nc.tensor.matmul(
                        ctx_ap(s),
                        ex[:, c, s * 128 : (s + 1) * 128],
                        val[:, c, :],
                        start=False,
                        stop=(c == NCK - 1),
                        skip_group_check=True,
                    )
                for c in range(14, NCK):
                    nc.tensor.matmul(
                        wr[:, s : s + 1],
                        ex[:, c, s * 128 : (s + 1) * 128],
                        ones_b[:],
                        start=False,
                        stop=(c == NCK - 1),
                        skip_group_check=True,
                    )

            def evict_half(h):
                for t in range(2):
                    for s in (4 * h + 2 * t, 4 * h + 2 * t + 1):
                        nc.vector.scalar_tensor_tensor(
                            out=out_sb[:, s, :],
                            in0=ctx_ap(s),
                            scalar=recip_sb[:, s : s + 1],
                            in1=bvr_sb[:, s % 4, :],
                            op0=mybir.AluOpType.mult,
                            op1=mybir.AluOpType.add,
                        )
                    s0 = 4 * h + 2 * t
                    nc.sync.dma_start(
                        out=out_view[:, s0 : s0 + 2, :],
                        in_=out_sb[:, s0 : s0 + 2, :],
                    )

            for s in range(4):
                finish_seg(s)
            nc.vector.reciprocal(out=recip_sb[:, 0:4], in_=wr[:, 0:4])
            evict_half(0)
            for s in range(4, 8):
                finish_seg(s)
            nc.vector.reciprocal(out=recip_sb[:, 4:8], in_=wr[:, 4:8])
            evict_half(1)

    nc.compile()
    return nc


def _prep_inputs(q, k, v, Wq, bq, Wk, bk, Wv, bv):
    """Host-side packing: bf16 cast + [s,h]->[h,s] transposes + weight pack."""
    import ml_dtypes

    bf16 = ml_dtypes.bfloat16
    q = np.asarray(q, dtype=np.float32)
    k = np.asarray(k, dtype=np.float32)
    v = np.asarray(v, dtype=np.float32)
    w_pack = (
        np.stack(
            [np.asarray(Wq, np.float32), np.asarray(Wk, np.float32),
             np.asarray(Wv, np.float32)], 0
        )
        .reshape(3, HC, 128, D)
        .transpose(2, 0, 1, 3)
        .reshape(128, 3 * HC * D)
        .astype(bf16)
    )
    w_pack = np.ascontiguousarray(w_pack)
    b_pack = np.ascontiguousarray(
        np.stack(
            [np.asarray(bq, np.float32), np.asarray(bk, np.float32),
             np.asarray(bv, np.float32)], 1
        )
    )
    bv_rep = np.ascontiguousarray(
        np.broadcast_to(np.asarray(bv, np.float32), (128, 4, D)).reshape(128, 4 * D)
    )

    half = S // 2
    in_maps = []
    for c in range(N_CORES):
        b_i, j = c // 2, c % 2
        in_maps.append(
            {
                "qt": np.ascontiguousarray(
                    q[b_i, j * half : (j + 1) * half].T.astype(bf16)
                ),
                "kt": np.ascontiguousarray(k[b_i].T.astype(bf16)),
                "vt": np.ascontiguousarray(v[b_i].T.astype(bf16)),
                "w": w_pack,
                "b": b_pack,
                "bvr": bv_rep,
            }
        )
    return in_maps


def kernel(q, k, v, Wq, bq, Wk, bk, Wv, bv):
    if "nc" not in _NC_CACHE:
        _NC_CACHE["nc"] = build()
    nc = _NC_CACHE["nc"]

    in_maps = _prep_inputs(q, k, v, Wq, bq, Wk, bk, Wv, bv)
    res = run_bass_kernel_spmd(nc, in_maps, list(range(N_CORES)))
    half = S // 2
    out = np.empty((B, S, D), dtype=np.float32)
    for c in range(N_CORES):
        b_i, j = c // 2, c % 2
        out[b_i, j * half : (j + 1) * half] = res.results[c]["out"]
    return out


# revision 6
# speedup vs baseline: 2.2827x; 1.0142x over previous
"""AttentionHead kernel for 8 TRN2 NeuronCores — v2 (no collectives).

Problem: q,k,v [4, 2048, 1024] f32; Wq/Wk/Wv [1024, 128]; out = softmax(
(qWq)(kWk)^T / sqrt(128)) @ (vWv)  -> [4, 2048, 128].

Sharding: core c = 2b+j owns batch b and query rows [1024j, 1024j+1024).
Instead of exchanging projected K/V halves between the two cores of a
batch (two 0.5MB AllGathers at ~28us each on the collective path), every
core projects the FULL K/V of its batch locally: the collective's fixed
cost dwarfs the extra 4MB of reads + ~7us of cheap bf16 projection
matmuls.

All activations are pre-transposed on the HOST (numpy) to [h, s] layout,
so no PE transposes / PSUM round-trips / DVE evictions are needed for
layout: the hidden dim is already on partitions for every projection
matmul.

On-chip dataflow (bf16 into the PE everywhere, fp32 PSUM accumulation):
  KT [d, sk]  = sum_c Wk-chunk.T @ ktT-chunk     (W stationary, kt moving)
  QT [d, sq]  likewise
  val[sk, d]  = sum_c vtT-chunk.T @ Wv-chunk     (vt stationary, W moving)
  scT[sk_c, sq] = KT-slice.T @ QT                (16 sk chunks)
  ex = exp(scale * scT)                          (ACT, bf16 out, kept in SBUF)
  ctx[sq_s, d] += ex-slice.T @ val[c]            (8 sq segs x 16 chunks)
  rs [sq_s, 1] += ex-slice.T @ ones              (rowsums, free-size-1 matmuls)
  out[sq_s, d] = ctx * (1/rs) + bv               (DVE/gpsimd eviction)

Q/K biases fold into the projection evictions (per-partition scalars on
d); the V bias commutes through the attention average and is added at
the output eviction (softmax rows sum to 1).  Junk "warm" matmuls plug
DMA-wait gaps in the PE stream so the p-state ramp never resets.
"""

import os
from contextlib import ExitStack

# The kernel needs jax's axon TRN2 backend; a pinned cpu-only platform list
# (used by some harnesses for the jax reference) would hide the devices.
if os.environ.get("JAX_PLATFORMS") not in (None, "", "axon"):
    del os.environ["JAX_PLATFORMS"]

import numpy as np

import concourse.bass as bass
import concourse.tile as tile
import concourse.mybir as mybir
from concourse import bacc
from concourse.bass_utils import run_bass_kernel_spmd

B, S, H, D = 4, 2048, 1024, 128
N_CORES = 8
SQ = 1024  # query rows per core
SK = 2048  # kv rows per batch (projected fully on both cores of the pair)
HC = H // 128  # 8 chunks of the hidden dim
NCK = SK // 128  # 16 sk chunks
NSEG = SQ // 128  # 8 sq segments
F32 = mybir.dt.float32
BF16 = mybir.dt.bfloat16
SCALE = 1.0 / float(np.sqrt(np.float32(D)))

_NC_CACHE = {}


def build():
    nc = bacc.Bacc(None, target_bir_lowering=False)
    qt_d = nc.declare_dram_parameter("qt", [H, SQ], BF16, isOutput=False)
    kt_d = nc.declare_dram_parameter("kt", [H, SK], BF16, isOutput=False)
    vt_d = nc.declare_dram_parameter("vt", [H, SK], BF16, isOutput=False)
    # packed weights: w[p, ((i, c, d))] = W_i[c*128 + p, d] for i in (q, k, v)
    w_d = nc.declare_dram_parameter("w", [128, 3 * HC * 128], BF16, isOutput=False)
    b_d = nc.declare_dram_parameter("b", [128, 3], F32, isOutput=False)
    bvr_d = nc.declare_dram_parameter("bvr", [128, 4 * 128], F32, isOutput=False)
    out_d = nc.declare_dram_parameter("out", [SQ, D], F32, isOutput=True)

    with tile.TileContext(nc) as tc, ExitStack() as top:
        const = top.enter_context(tc.tile_pool(name="const", bufs=1))
        w_sb = const.tile([128, 3 * HC * 128], BF16)
        b_sb = const.tile([128, 3], F32)
        bvr_sb = const.tile([128, 4, 128], F32)
        ones_b = const.tile([128, 1], BF16)
        warm_w = const.tile([128, 128], BF16)
        warm_a = const.tile([128, 384], BF16)
        nc.vector.memset(ones_b[:], 1.0)
        nc.vector.memset(warm_w[:], 0.0)
        nc.vector.memset(warm_a[:], 0.0)

        def wsl(i, c):  # stationary weight slice [h-chunk, d]
            return w_sb[:, (i * HC + c) * 128 : (i * HC + c + 1) * 128]

        xin = top.enter_context(tc.tile_pool(name="xin", bufs=1))
        qt_sb = xin.tile([128, HC, SQ], BF16)
        kt_sb = xin.tile([128, HC, SK], BF16)
        vt_sb = xin.tile([128, HC, SK], BF16)

        res = top.enter_context(tc.tile_pool(name="res", bufs=1))
        QT = res.tile([128, SQ], BF16)  # [d, sq]
        KT = res.tile([128, SK], BF16)  # [d, sk]
        val = res.tile([128, NCK, 128], BF16)  # [sk-in-chunk, c, d]
        ex = res.tile([128, NCK, SQ], BF16)  # [sk-in-chunk, c, sq]
        recip_sb = res.tile([128, NSEG], F32)
        out_sb = res.tile([128, NSEG, 128], F32)

        # ---- input DMAs (SP/hwdge), ordered for pipeline liveness:
        # everything feeding the ACT-paced scores/exp stream (wk, kt, wq, qt)
        # goes first; vt only feeds PE-local context work and loads last.
        def load_w(i):
            sl = slice(i * HC * 128, (i + 1) * HC * 128)
            nc.sync.dma_start(out=w_sb[:, sl], in_=w_d[:, sl])

        def load_slab(dst, src, s0, s1):
            nc.sync.dma_start(
                out=dst[:, :, s0:s1],
                in_=src[:, s0:s1].rearrange("(c p) s -> p c s", p=128),
            )

        nc.sync.dma_start(out=w_sb[:, 0 : 2 * HC * 128], in_=w_d[:, 0 : 2 * HC * 128])
        nc.sync.dma_start(out=b_sb[:], in_=b_d[:])
        for i in range(4):
            load_slab(qt_sb, qt_d, i * 256, (i + 1) * 256)
        load_slab(kt_sb, kt_d, 0, 256)
        load_slab(kt_sb, kt_d, 256, 512)
        nc.sync.dma_start(
            out=w_sb[:, 2 * HC * 128 :], in_=w_d[:, 2 * HC * 128 :]
        )  # wv
        load_slab(kt_sb, kt_d, 512, 768)
        load_slab(kt_sb, kt_d, 768, 1024)
        load_slab(vt_sb, vt_d, 0, 256)
        load_slab(vt_sb, vt_d, 256, 512)
        load_slab(kt_sb, kt_d, 1024, 1280)
        load_slab(kt_sb, kt_d, 1280, 1536)
        load_slab(kt_sb, kt_d, 1536, 1792)
        load_slab(kt_sb, kt_d, 1792, 2048)
        load_slab(vt_sb, vt_d, 512, 768)
        load_slab(vt_sb, vt_d, 768, 1024)
        load_slab(vt_sb, vt_d, 1024, 1280)
        load_slab(vt_sb, vt_d, 1280, 1536)
        load_slab(vt_sb, vt_d, 1536, 1792)
        load_slab(vt_sb, vt_d, 1792, 2048)
        nc.sync.dma_start(
            out=bvr_sb[:], in_=bvr_d[:].rearrange("p (s d) -> p s d", d=128)
        )

        with ExitStack() as ph:
            # PSUM budget (8 banks, bank-granular tiles):
            # pj 1 + sc 2x2 + ctx 2 + wr 1 (rs accumulator cols 0:8, junk
            # warm-up matmul region cols 128:384 — disjoint has_written
            # ranges in one bank).
            pj_ps = ph.enter_context(tc.tile_pool(name="pj_ps", bufs=1, space="PSUM"))
            sc_ps = ph.enter_context(tc.tile_pool(name="sc_ps", bufs=2, space="PSUM"))
            ctx_ps = ph.enter_context(tc.tile_pool(name="ctx_ps", bufs=1, space="PSUM"))
            wr_ps = ph.enter_context(tc.tile_pool(name="wr_ps", bufs=1, space="PSUM"))

            # two independent bank-tiles so tail evictions of the first
            # half overlap the PE finishing the second half
            ctxA = ctx_ps.tile([128, SQ // 2], F32)  # segs 0-3
            ctxB = ctx_ps.tile([128, SQ // 2], F32)  # segs 4-7
            wr = wr_ps.tile([128, 512], F32)  # rowsum accum cols 0:8

            def ctx_ap(s):
                t = ctxA if s < 4 else ctxB
                return t[:, (s % 4) * 128 : (s % 4 + 1) * 128]

            def warm(n):
                # junk matmuls: keep the PE busy through DMA waits so the
                # p-state ramp (3us to full clock) never restarts.
                for _ in range(n):
                    nc.tensor.matmul(
                        wr[:, 128:384], warm_w[:], warm_a[:, 0:256],
                        start=True, stop=True, skip_group_check=True,
                    )

            pgi = [0]

            def project_g(dst, wi, src, g, bias):
                # alternate between the pj bank and the spare region of the
                # wr bank so group g+1 accumulates while group g evicts (a
                # single bank would serialize every group behind its
                # eviction).  All projection start=True marks land before the
                # rowsum epoch opens (PE is in-order), so sharing wr is safe.
                i = pgi[0]
                pgi[0] += 1
                if i % 2 == 0:
                    pjt = pj_ps.tile([128, 256], F32, tag="pj", name=f"pj{i}")
                    pj = pjt[:]
                else:
                    pj = wr[:, 128:384]
                for c in range(HC):
                    nc.tensor.matmul(
                        pj,
                        wsl(wi, c),
                        src[:, c, g * 256 : (g + 1) * 256],
                        start=(c == 0),
                        stop=(c == HC - 1),
                        skip_group_check=True,
                    )
                eng = nc.vector
                eng.tensor_scalar(
                    out=dst[:, g * 256 : (g + 1) * 256], in0=pj, scalar1=bias,
                    scalar2=None, op0=mybir.AluOpType.add,
                )

            def vproj_pair(p, bank="pj"):  # sk chunks 2p, 2p+1
                # start=True zeroes the whole 2KB PSUM bank (zero region), so
                # only the FIRST matmul touching the bank starts; stop only on
                # the last.  Untouched-but-started bytes zero lazily on first
                # write (per-element has_written), so si=1 accumulates
                # correctly with start=False.  Pairs alternate between the pj
                # bank and the wr spare region so pair p+1 accumulates while
                # pair p evicts; all rowsum matmuls are emitted after the
                # last wr-bank start (PE is in-order), so sharing wr is safe.
                if bank == "pj":
                    vpt = pj_ps.tile([128, 256], F32, tag="pj", name=f"vp{p}")
                    vp = vpt[:]
                elif bank == "wr":
                    vp = wr[:, 128:384]
                else:  # recycle a scores bank (scores for this buf are done)
                    vpt = sc_ps.tile([128, SQ], F32, tag="sc", name=f"vp{p}")
                    vp = vpt[:, 0:256]
                for si in range(2):
                    seg = 2 * p + si
                    for c in range(HC):
                        nc.tensor.matmul(
                            vp[:, si * 128 : (si + 1) * 128],
                            vt_sb[:, c, seg * 128 : (seg + 1) * 128],
                            wsl(2, c),
                            start=(si == 0 and c == 0),
                            stop=(si == 1 and c == HC - 1),
                            skip_group_check=True,
                        )
                nc.vector.tensor_copy(out=val[:, 2 * p : 2 * p + 2, :], in_=vp)

            def scores(c):
                sc = sc_ps.tile([128, SQ], F32, tag="sc")
                for h in range(2):
                    nc.tensor.matmul(
                        sc[:, h * 512 : (h + 1) * 512],
                        KT[:, c * 128 : (c + 1) * 128],
                        QT[:, h * 512 : (h + 1) * 512],
                        start=True,
                        stop=True,
                    )
                nc.scalar.activation(
                    ex[:, c, :], sc[:], mybir.ActivationFunctionType.Exp, scale=SCALE
                )

            def ctx_chunk(c, first, last):
                # One start per 2KB PSUM bank per epoch (see vproj_pair): the
                # ctx tile spans 2 banks (segs 0-3 / 4-7).
                for s in range(NSEG):
                    nc.tensor.matmul(
                        ctx_ap(s),
                        ex[:, c, s * 128 : (s + 1) * 128],
                        val[:, c, :],
                        start=first and s % 4 == 0,
                        stop=last,
                        skip_group_check=True,
                    )

            def rs_chunk(c, first, last):
                # rowsums: free-size-1 matmuls, engine-time-free; deferred
                # until after the final wr-bank projection start.
                for s in range(NSEG):
                    nc.tensor.matmul(
                        wr[:, s : s + 1],
                        ex[:, c, s * 128 : (s + 1) * 128],
                        ones_b[:],
                        start=first and s == 0,
                        stop=last,
                        skip_group_check=True,
                    )

            # ---- PE stream (in emission order) ----
            # Interleave tuned to slab arrival: kproj groups right behind
            # their kt slabs, vproj pairs behind their vt slabs, ctx chunks
            # trailing the corresponding val evictions, scores ACT-paced.
            # wr's spare region hosts odd projection groups and vproj p1/p3/
            # p5; every wr start precedes the rowsum epoch (PE is in-order).
            warm(14)
            project_g(QT, 0, qt_sb, 0, b_sb[:, 0:1])
            warm(3)
            project_g(QT, 0, qt_sb, 1, b_sb[:, 0:1])
            warm(3)
            project_g(QT, 0, qt_sb, 2, b_sb[:, 0:1])
            warm(3)
            project_g(QT, 0, qt_sb, 3, b_sb[:, 0:1])
            warm(3)
            project_g(KT, 1, kt_sb, 0, b_sb[:, 1:2])
            warm(3)
            project_g(KT, 1, kt_sb, 1, b_sb[:, 1:2])
            for c in range(4):
                scores(c)
            project_g(KT, 1, kt_sb, 2, b_sb[:, 1:2])
            project_g(KT, 1, kt_sb, 3, b_sb[:, 1:2])
            scores(4)
            scores(5)
            vproj_pair(0, bank="pj")
            vproj_pair(1, bank="wr")
            ctx_chunk(0, first=True, last=False)
            scores(6)
            ctx_chunk(1, first=False, last=False)
            scores(7)
            project_g(KT, 1, kt_sb, 4, b_sb[:, 1:2])
            project_g(KT, 1, kt_sb, 5, b_sb[:, 1:2])
            ctx_chunk(2, first=False, last=False)
            scores(8)
            ctx_chunk(3, first=False, last=False)
            scores(9)
            project_g(KT, 1, kt_sb, 6, b_sb[:, 1:2])
            project_g(KT, 1, kt_sb, 7, b_sb[:, 1:2])
            scores(10)
            scores(11)
            vproj_pair(2, bank="pj")
            vproj_pair(3, bank="wr")
            scores(12)
            scores(13)
            scores(14)
            scores(15)
            vproj_pair(4, bank="pj")
            vproj_pair(5, bank="wr")  # last wr-bank start
            for c in range(4, 10):
                ctx_chunk(c, first=False, last=False)
            for c in range(14):
                rs_chunk(c, first=(c == 0), last=False)
            ctx_chunk(10, first=False, last=False)
            ctx_chunk(11, first=False, last=False)
            vproj_pair(6, bank="pj")
            vproj_pair(7, bank="sc")

            # ---- tail, seg-major: finish bank-A segs (chunks 12-15 + final
            # rowsums) first so their reciprocal/normalize/store chain runs
            # while the PE is still accumulating bank-B segs ----
            out_view = out_d[:].rearrange("(s p) d -> p s d", p=128)

            def finish_seg(s):
                for c in range(12, NCK):
                    nc.tensor.matmul(
                        ctx_ap(s),
                        ex[:, c, s * 128 : (s + 1) * 128],
                        val[:, c, :],
                        start=False,
                        stop=(c == NCK - 1),
                        skip_group_check=True,
                    )
                for c in range(14, NCK):
                    nc.tensor.matmul(
                        wr[:, s : s + 1],
                        ex[:, c, s * 128 : (s + 1) * 128],
                        ones_b[:],
                        start=False,
                        stop=(c == NCK - 1),
                        skip_group_check=True,
                    )

            def evict_half(h):
                for t in range(2):
                    for s in (4 * h + 2 * t, 4 * h + 2 * t + 1):
                        nc.vector.scalar_tensor_tensor(
                            out=out_sb[:, s, :],
                            in0=ctx_ap(s),
                            scalar=recip_sb[:, s : s + 1],
                            in1=bvr_sb[:, s % 4, :],
                            op0=mybir.AluOpType.mult,
                            op1=mybir.AluOpType.add,
                        )
                    s0 = 4 * h + 2 * t
                    nc.sync.dma_start(
                        out=out_view[:, s0 : s0 + 2, :],
                        in_=out_sb[:, s0 : s0 + 2, :],
                    )

            for t in range(4):
                finish_seg(2 * t)
                finish_seg(2 * t + 1)
                nc.vector.reciprocal(
                    out=recip_sb[:, 2 * t : 2 * t + 2], in_=wr[:, 2 * t : 2 * t + 2]
                )
                for s in (2 * t, 2 * t + 1):
                    nc.vector.scalar_tensor_tensor(
                        out=out_sb[:, s, :],
                        in0=ctx_ap(s),
                        scalar=recip_sb[:, s : s + 1],
                        in1=bvr_sb[:, s % 4, :],
                        op0=mybir.AluOpType.mult,
                        op1=mybir.AluOpType.add,
                    )
                nc.sync.dma_start(
                    out=out_view[:, 2 * t : 2 * t + 2, :],
                    in_=out_sb[:, 2 * t : 2 * t + 2, :],
                )

    nc.compile()
    return nc


def _prep_inputs(q, k, v, Wq, bq, Wk, bk, Wv, bv):
    """Host-side packing: bf16 cast + [s,h]->[h,s] transposes + weight pack."""
    import ml_dtypes

    bf16 = ml_dtypes.bfloat16
    q = np.asarray(q, dtype=np.float32)
    k = np.asarray(k, dtype=np.float32)
    v = np.asarray(v, dtype=np.float32)
    w_pack = (
        np.stack(
            [np.asarray(Wq, np.float32), np.asarray(Wk, np.float32),
             np.asarray(Wv, np.float32)], 0
        )
        .reshape(3, HC, 128, D)
        .transpose(2, 0, 1, 3)
        .reshape(128, 3 * HC * D)
        .astype(bf16)
    )
    w_pack = np.ascontiguousarray(w_pack)
    b_pack = np.ascontiguousarray(
        np.stack(
            [np.asarray(bq, np.float32), np.asarray(bk, np.float32),
             np.asarray(bv, np.float32)], 1
        )
    )
    bv_rep = np.ascontiguousarray(
        np.broadcast_to(np.asarray(bv, np.float32), (128, 4, D)).reshape(128, 4 * D)
    )

    half = S // 2
    in_maps = []
    for c in range(N_CORES):
        b_i, j = c // 2, c % 2
        in_maps.append(
            {
                "qt": np.ascontiguousarray(
                    q[b_i, j * half : (j + 1) * half].T.astype(bf16)
                ),
                "kt": np.ascontiguousarray(k[b_i].T.astype(bf16)),
                "vt": np.ascontiguousarray(v[b_i].T.astype(bf16)),
                "w": w_pack,
                "b": b_pack,
                "bvr": bv_rep,
            }
        )
    return in_maps


def kernel(q, k, v, Wq, bq, Wk, bk, Wv, bv):
    if "nc" not in _NC_CACHE:
        _NC_CACHE["nc"] = build()
    nc = _NC_CACHE["nc"]

    in_maps = _prep_inputs(q, k, v, Wq, bq, Wk, bk, Wv, bv)
    res = run_bass_kernel_spmd(nc, in_maps, list(range(N_CORES)))
    half = S // 2
    out = np.empty((B, S, D), dtype=np.float32)
    for c in range(N_CORES):
        b_i, j = c // 2, c % 2
        out[b_i, j * half : (j + 1) * half] = res.results[c]["out"]
    return out


# revision 7
# speedup vs baseline: 2.2930x; 1.0045x over previous
"""AttentionHead kernel for 8 TRN2 NeuronCores — v2 (no collectives).

Problem: q,k,v [4, 2048, 1024] f32; Wq/Wk/Wv [1024, 128]; out = softmax(
(qWq)(kWk)^T / sqrt(128)) @ (vWv)  -> [4, 2048, 128].

Sharding: core c = 2b+j owns batch b and query rows [1024j, 1024j+1024).
Instead of exchanging projected K/V halves between the two cores of a
batch (two 0.5MB AllGathers at ~28us each on the collective path), every
core projects the FULL K/V of its batch locally: the collective's fixed
cost dwarfs the extra 4MB of reads + ~7us of cheap bf16 projection
matmuls.

All activations are pre-transposed on the HOST (numpy) to [h, s] layout,
so no PE transposes / PSUM round-trips / DVE evictions are needed for
layout: the hidden dim is already on partitions for every projection
matmul.

On-chip dataflow (bf16 into the PE everywhere, fp32 PSUM accumulation):
  KT [d, sk]  = sum_c Wk-chunk.T @ ktT-chunk     (W stationary, kt moving)
  QT [d, sq]  likewise
  val[sk, d]  = sum_c vtT-chunk.T @ Wv-chunk     (vt stationary, W moving)
  scT[sk_c, sq] = KT-slice.T @ QT                (16 sk chunks)
  ex = exp(scale * scT)                          (ACT, bf16 out, kept in SBUF)
  ctx[sq_s, d] += ex-slice.T @ val[c]            (8 sq segs x 16 chunks)
  rs [sq_s, 1] += ex-slice.T @ ones              (rowsums, free-size-1 matmuls)
  out[sq_s, d] = ctx * (1/rs) + bv               (DVE/gpsimd eviction)

Q/K biases fold into the projection evictions (per-partition scalars on
d); the V bias commutes through the attention average and is added at
the output eviction (softmax rows sum to 1).  Junk "warm" matmuls plug
DMA-wait gaps in the PE stream so the p-state ramp never resets.
"""

import os
from contextlib import ExitStack

# The kernel needs jax's axon TRN2 backend; a pinned cpu-only platform list
# (used by some harnesses for the jax reference) would hide the devices.
if os.environ.get("JAX_PLATFORMS") not in (None, "", "axon"):
    del os.environ["JAX_PLATFORMS"]

import numpy as np

import concourse.bass as bass
import concourse.tile as tile
import concourse.mybir as mybir
from concourse import bacc
from concourse.bass_utils import run_bass_kernel_spmd

B, S, H, D = 4, 2048, 1024, 128
N_CORES = 8
SQ = 1024  # query rows per core
SK = 2048  # kv rows per batch (projected fully on both cores of the pair)
HC = H // 128  # 8 chunks of the hidden dim
NCK = SK // 128  # 16 sk chunks
NSEG = SQ // 128  # 8 sq segments
F32 = mybir.dt.float32
BF16 = mybir.dt.bfloat16
SCALE = 1.0 / float(np.sqrt(np.float32(D)))

_NC_CACHE = {}


def build():
    nc = bacc.Bacc(None, target_bir_lowering=False)
    qt_d = nc.declare_dram_parameter("qt", [H, SQ], BF16, isOutput=False)
    kt_d = nc.declare_dram_parameter("kt", [H, SK], BF16, isOutput=False)
    vt_d = nc.declare_dram_parameter("vt", [H, SK], BF16, isOutput=False)
    # packed weights: w[p, ((i, c, d))] = W_i[c*128 + p, d] for i in (q, k, v)
    w_d = nc.declare_dram_parameter("w", [128, 3 * HC * 128], BF16, isOutput=False)
    b_d = nc.declare_dram_parameter("b", [128, 3], F32, isOutput=False)
    bvr_d = nc.declare_dram_parameter("bvr", [128, 4 * 128], F32, isOutput=False)
    out_d = nc.declare_dram_parameter("out", [SQ, D], F32, isOutput=True)

    with tile.TileContext(nc) as tc, ExitStack() as top:
        const = top.enter_context(tc.tile_pool(name="const", bufs=1))
        w_sb = const.tile([128, 3 * HC * 128], BF16)
        b_sb = const.tile([128, 3], F32)
        bvr_sb = const.tile([128, 4, 128], F32)
        ones_b = const.tile([128, 1], BF16)
        warm_w = const.tile([128, 128], BF16)
        warm_a = const.tile([128, 384], BF16)
        nc.vector.memset(ones_b[:], 1.0)
        nc.vector.memset(warm_w[:], 0.0)
        nc.vector.memset(warm_a[:], 0.0)

        def wsl(i, c):  # stationary weight slice [h-chunk, d]
            return w_sb[:, (i * HC + c) * 128 : (i * HC + c + 1) * 128]

        xin = top.enter_context(tc.tile_pool(name="xin", bufs=1))
        qt_sb = xin.tile([128, HC, SQ], BF16)
        kt_sb = xin.tile([128, HC, SK], BF16)
        vt_sb = xin.tile([128, HC, SK], BF16)

        res = top.enter_context(tc.tile_pool(name="res", bufs=1))
        QT = res.tile([128, SQ], BF16)  # [d, sq]
        KT = res.tile([128, SK], BF16)  # [d, sk]
        val = res.tile([128, NCK, 128], BF16)  # [sk-in-chunk, c, d]
        ex = res.tile([128, NCK, SQ], BF16)  # [sk-in-chunk, c, sq]
        recip_sb = res.tile([128, NSEG], F32)
        out_sb = res.tile([128, NSEG, 128], F32)

        # ---- input DMAs (SP/hwdge), ordered for pipeline liveness:
        # everything feeding the ACT-paced scores/exp stream (wk, kt, wq, qt)
        # goes first; vt only feeds PE-local context work and loads last.
        def load_w(i):
            sl = slice(i * HC * 128, (i + 1) * HC * 128)
            nc.sync.dma_start(out=w_sb[:, sl], in_=w_d[:, sl])

        def load_slab(dst, src, s0, s1):
            nc.sync.dma_start(
                out=dst[:, :, s0:s1],
                in_=src[:, s0:s1].rearrange("(c p) s -> p c s", p=128),
            )

        nc.sync.dma_start(out=w_sb[:, 0 : 2 * HC * 128], in_=w_d[:, 0 : 2 * HC * 128])
        nc.sync.dma_start(out=b_sb[:], in_=b_d[:])
        for i in range(4):
            load_slab(qt_sb, qt_d, i * 256, (i + 1) * 256)
        load_slab(kt_sb, kt_d, 0, 256)
        load_slab(kt_sb, kt_d, 256, 512)
        nc.sync.dma_start(
            out=w_sb[:, 2 * HC * 128 :], in_=w_d[:, 2 * HC * 128 :]
        )  # wv
        load_slab(kt_sb, kt_d, 512, 768)
        load_slab(kt_sb, kt_d, 768, 1024)
        load_slab(vt_sb, vt_d, 0, 256)
        load_slab(vt_sb, vt_d, 256, 512)
        load_slab(kt_sb, kt_d, 1024, 1280)
        load_slab(kt_sb, kt_d, 1280, 1536)
        load_slab(kt_sb, kt_d, 1536, 1792)
        load_slab(kt_sb, kt_d, 1792, 2048)
        load_slab(vt_sb, vt_d, 512, 768)
        load_slab(vt_sb, vt_d, 768, 1024)
        load_slab(vt_sb, vt_d, 1024, 1280)
        load_slab(vt_sb, vt_d, 1280, 1536)
        load_slab(vt_sb, vt_d, 1536, 1792)
        load_slab(vt_sb, vt_d, 1792, 2048)
        nc.sync.dma_start(
            out=bvr_sb[:], in_=bvr_d[:].rearrange("p (s d) -> p s d", d=128)
        )

        with ExitStack() as ph:
            # PSUM budget (8 banks, bank-granular tiles):
            # pj 1 + sc 2x2 + ctx 2 + wr 1 (rs accumulator cols 0:8, junk
            # warm-up matmul region cols 128:384 — disjoint has_written
            # ranges in one bank).
            pj_ps = ph.enter_context(tc.tile_pool(name="pj_ps", bufs=1, space="PSUM"))
            sc_ps = ph.enter_context(tc.tile_pool(name="sc_ps", bufs=2, space="PSUM"))
            ctx_ps = ph.enter_context(tc.tile_pool(name="ctx_ps", bufs=1, space="PSUM"))
            wr_ps = ph.enter_context(tc.tile_pool(name="wr_ps", bufs=1, space="PSUM"))

            # two independent bank-tiles so tail evictions of the first
            # half overlap the PE finishing the second half
            ctxA = ctx_ps.tile([128, SQ // 2], F32)  # segs 0-3
            ctxB = ctx_ps.tile([128, SQ // 2], F32)  # segs 4-7
            wr = wr_ps.tile([128, 512], F32)  # rowsum accum cols 0:8

            def ctx_ap(s):
                t = ctxA if s < 4 else ctxB
                return t[:, (s % 4) * 128 : (s % 4 + 1) * 128]

            def warm(n):
                # junk matmuls: keep the PE busy through DMA waits so the
                # p-state ramp (3us to full clock) never restarts.
                for _ in range(n):
                    nc.tensor.matmul(
                        wr[:, 128:384], warm_w[:], warm_a[:, 0:256],
                        start=True, stop=True, skip_group_check=True,
                    )

            pgi = [0]

            def project_g(dst, wi, src, g, bias):
                # alternate between the pj bank and the spare region of the
                # wr bank so group g+1 accumulates while group g evicts (a
                # single bank would serialize every group behind its
                # eviction).  All projection start=True marks land before the
                # rowsum epoch opens (PE is in-order), so sharing wr is safe.
                i = pgi[0]
                pgi[0] += 1
                if i % 2 == 0:
                    pjt = pj_ps.tile([128, 256], F32, tag="pj", name=f"pj{i}")
                    pj = pjt[:]
                else:
                    pj = wr[:, 128:384]
                for c in range(HC):
                    nc.tensor.matmul(
                        pj,
                        wsl(wi, c),
                        src[:, c, g * 256 : (g + 1) * 256],
                        start=(c == 0),
                        stop=(c == HC - 1),
                        skip_group_check=True,
                    )
                eng = nc.vector
                eng.tensor_scalar(
                    out=dst[:, g * 256 : (g + 1) * 256], in0=pj, scalar1=bias,
                    scalar2=None, op0=mybir.AluOpType.add,
                )

            def vproj_pair(p, bank="pj"):  # sk chunks 2p, 2p+1
                # start=True zeroes the whole 2KB PSUM bank (zero region), so
                # only the FIRST matmul touching the bank starts; stop only on
                # the last.  Untouched-but-started bytes zero lazily on first
                # write (per-element has_written), so si=1 accumulates
                # correctly with start=False.  Pairs alternate between the pj
                # bank and the wr spare region so pair p+1 accumulates while
                # pair p evicts; all rowsum matmuls are emitted after the
                # last wr-bank start (PE is in-order), so sharing wr is safe.
                if bank == "pj":
                    vpt = pj_ps.tile([128, 256], F32, tag="pj", name=f"vp{p}")
                    vp = vpt[:]
                elif bank == "wr":
                    vp = wr[:, 128:384]
                else:  # recycle a scores bank (scores for this buf are done)
                    vpt = sc_ps.tile([128, SQ], F32, tag="sc", name=f"vp{p}")
                    vp = vpt[:, 0:256]
                for si in range(2):
                    seg = 2 * p + si
                    for c in range(HC):
                        nc.tensor.matmul(
                            vp[:, si * 128 : (si + 1) * 128],
                            vt_sb[:, c, seg * 128 : (seg + 1) * 128],
                            wsl(2, c),
                            start=(si == 0 and c == 0),
                            stop=(si == 1 and c == HC - 1),
                            skip_group_check=True,
                        )
                nc.vector.tensor_copy(out=val[:, 2 * p : 2 * p + 2, :], in_=vp)

            def scores(c):
                sc = sc_ps.tile([128, SQ], F32, tag="sc")
                for h in range(2):
                    nc.tensor.matmul(
                        sc[:, h * 512 : (h + 1) * 512],
                        KT[:, c * 128 : (c + 1) * 128],
                        QT[:, h * 512 : (h + 1) * 512],
                        start=True,
                        stop=True,
                    )
                nc.scalar.activation(
                    ex[:, c, :], sc[:], mybir.ActivationFunctionType.Exp, scale=SCALE
                )

            def ctx_chunk(c, first, last):
                # One start per 2KB PSUM bank per epoch (see vproj_pair): the
                # ctx tile spans 2 banks (segs 0-3 / 4-7).
                for s in range(NSEG):
                    nc.tensor.matmul(
                        ctx_ap(s),
                        ex[:, c, s * 128 : (s + 1) * 128],
                        val[:, c, :],
                        start=first and s % 4 == 0,
                        stop=last,
                        skip_group_check=True,
                    )

            def rs_chunk(c, first, last):
                # rowsums: free-size-1 matmuls, engine-time-free; deferred
                # until after the final wr-bank projection start.
                for s in range(NSEG):
                    nc.tensor.matmul(
                        wr[:, s : s + 1],
                        ex[:, c, s * 128 : (s + 1) * 128],
                        ones_b[:],
                        start=first and s == 0,
                        stop=last,
                        skip_group_check=True,
                    )

            # ---- PE stream (in emission order) ----
            # Interleave tuned to slab arrival: kproj groups right behind
            # their kt slabs, vproj pairs behind their vt slabs, ctx chunks
            # trailing the corresponding val evictions, scores ACT-paced.
            # wr's spare region hosts odd projection groups and vproj p1/p3/
            # p5; every wr start precedes the rowsum epoch (PE is in-order).
            warm(14)
            project_g(QT, 0, qt_sb, 0, b_sb[:, 0:1])
            warm(3)
            project_g(QT, 0, qt_sb, 1, b_sb[:, 0:1])
            warm(3)
            project_g(QT, 0, qt_sb, 2, b_sb[:, 0:1])
            warm(3)
            project_g(QT, 0, qt_sb, 3, b_sb[:, 0:1])
            warm(3)
            project_g(KT, 1, kt_sb, 0, b_sb[:, 1:2])
            warm(3)
            project_g(KT, 1, kt_sb, 1, b_sb[:, 1:2])
            for c in range(4):
                scores(c)
            project_g(KT, 1, kt_sb, 2, b_sb[:, 1:2])
            project_g(KT, 1, kt_sb, 3, b_sb[:, 1:2])
            scores(4)
            scores(5)
            vproj_pair(0, bank="pj")
            vproj_pair(1, bank="wr")
            ctx_chunk(0, first=True, last=False)
            scores(6)
            ctx_chunk(1, first=False, last=False)
            scores(7)
            project_g(KT, 1, kt_sb, 4, b_sb[:, 1:2])
            project_g(KT, 1, kt_sb, 5, b_sb[:, 1:2])
            ctx_chunk(2, first=False, last=False)
            scores(8)
            ctx_chunk(3, first=False, last=False)
            scores(9)
            project_g(KT, 1, kt_sb, 6, b_sb[:, 1:2])
            project_g(KT, 1, kt_sb, 7, b_sb[:, 1:2])
            scores(10)
            scores(11)
            vproj_pair(2, bank="pj")
            vproj_pair(3, bank="wr")
            scores(12)
            scores(13)
            scores(14)
            scores(15)
            vproj_pair(4, bank="pj")
            vproj_pair(5, bank="wr")  # last wr-bank start
            for c in range(4, 10):
                ctx_chunk(c, first=False, last=False)
            for c in range(14):
                rs_chunk(c, first=(c == 0), last=False)
            ctx_chunk(10, first=False, last=False)
            ctx_chunk(11, first=False, last=False)
            vproj_pair(6, bank="pj")
            vproj_pair(7, bank="sc")

            # ---- tail, seg-major: finish bank-A segs (chunks 12-15 + final
            # rowsums) first so their reciprocal/normalize/store chain runs
            # while the PE is still accumulating bank-B segs ----
            out_view = out_d[:].rearrange("(s p) d -> p s d", p=128)

            def finish_rs(s):
                for c in range(14, NCK):
                    nc.tensor.matmul(
                        wr[:, s : s + 1],
                        ex[:, c, s * 128 : (s + 1) * 128],
                        ones_b[:],
                        start=False,
                        stop=(c == NCK - 1),
                        skip_group_check=True,
                    )

            def finish_ctx(s):
                for c in range(12, NCK):
                    nc.tensor.matmul(
                        ctx_ap(s),
                        ex[:, c, s * 128 : (s + 1) * 128],
                        val[:, c, :],
                        start=False,
                        stop=(c == NCK - 1),
                        skip_group_check=True,
                    )

            def evict_half(h):
                for t in range(2):
                    for s in (4 * h + 2 * t, 4 * h + 2 * t + 1):
                        nc.vector.scalar_tensor_tensor(
                            out=out_sb[:, s, :],
                            in0=ctx_ap(s),
                            scalar=recip_sb[:, s : s + 1],
                            in1=bvr_sb[:, s % 4, :],
                            op0=mybir.AluOpType.mult,
                            op1=mybir.AluOpType.add,
                        )
                    s0 = 4 * h + 2 * t
                    nc.sync.dma_start(
                        out=out_view[:, s0 : s0 + 2, :],
                        in_=out_sb[:, s0 : s0 + 2, :],
                    )

            # rowsum stops first (settles the wr bank), then reciprocals
            # (no later wr writes -> no bank-WAR), then per-quarter context
            # stops with their evict+store chains draining behind the PE
            for s in range(NSEG):
                finish_rs(s)
            nc.vector.reciprocal(out=recip_sb[:, 0:4], in_=wr[:, 0:4])
            nc.vector.reciprocal(out=recip_sb[:, 4:8], in_=wr[:, 4:8])
            for t in range(4):
                for s in (2 * t, 2 * t + 1):
                    finish_ctx(s)
                for s in (2 * t, 2 * t + 1):
                    nc.vector.scalar_tensor_tensor(
                        out=out_sb[:, s, :],
                        in0=ctx_ap(s),
                        scalar=recip_sb[:, s : s + 1],
                        in1=bvr_sb[:, s % 4, :],
                        op0=mybir.AluOpType.mult,
                        op1=mybir.AluOpType.add,
                    )
                nc.sync.dma_start(
                    out=out_view[:, 2 * t : 2 * t + 2, :],
                    in_=out_sb[:, 2 * t : 2 * t + 2, :],
                )

    nc.compile()
    return nc


def _prep_inputs(q, k, v, Wq, bq, Wk, bk, Wv, bv):
    """Host-side packing: bf16 cast + [s,h]->[h,s] transposes + weight pack."""
    import ml_dtypes

    bf16 = ml_dtypes.bfloat16
    q = np.asarray(q, dtype=np.float32)
    k = np.asarray(k, dtype=np.float32)
    v = np.asarray(v, dtype=np.float32)
    w_pack = (
        np.stack(
            [np.asarray(Wq, np.float32), np.asarray(Wk, np.float32),
             np.asarray(Wv, np.float32)], 0
        )
        .reshape(3, HC, 128, D)
        .transpose(2, 0, 1, 3)
        .reshape(128, 3 * HC * D)
        .astype(bf16)
    )
    w_pack = np.ascontiguousarray(w_pack)
    b_pack = np.ascontiguousarray(
        np.stack(
            [np.asarray(bq, np.float32), np.asarray(bk, np.float32),
             np.asarray(bv, np.float32)], 1
        )
    )
    bv_rep = np.ascontiguousarray(
        np.broadcast_to(np.asarray(bv, np.float32), (128, 4, D)).reshape(128, 4 * D)
    )

    half = S // 2
    in_maps = []
    for c in range(N_CORES):
        b_i, j = c // 2, c % 2
        in_maps.append(
            {
                "qt": np.ascontiguousarray(
                    q[b_i, j * half : (j + 1) * half].T.astype(bf16)
                ),
                "kt": np.ascontiguousarray(k[b_i].T.astype(bf16)),
                "vt": np.ascontiguousarray(v[b_i].T.astype(bf16)),
                "w": w_pack,
                "b": b_pack,
                "bvr": bv_rep,
            }
        )
    return in_maps


def kernel(q, k, v, Wq, bq, Wk, bk, Wv, bv):
    if "nc" not in _NC_CACHE:
        _NC_CACHE["nc"] = build()
    nc = _NC_CACHE["nc"]

    in_maps = _prep_inputs(q, k, v, Wq, bq, Wk, bk, Wv, bv)
    res = run_bass_kernel_spmd(nc, in_maps, list(range(N_CORES)))
    half = S // 2
    out = np.empty((B, S, D), dtype=np.float32)
    for c in range(N_CORES):
        b_i, j = c // 2, c % 2
        out[b_i, j * half : (j + 1) * half] = res.results[c]["out"]
    return out
